# revision 14
# baseline (speedup 1.0000x reference)
"""GCN (3x GCNConv + BN + residual, mean-pool, MLP head) on 8 trn2 NeuronCores.

Sharding: nodes split contiguously across 8 cores (6250 each); each core owns
the edges whose TARGET lands in its shard (plus self-loops). Per layer, each
core aggregates source features over its incident edges (GCN normalization is
linear, so aggregate-then-transform), applies the folded linear+BN epilogue,
and the bf16 activations are AllGathered (split in two halves so the first
half overlaps the tail of the layer) so every core can gather arbitrary
source rows next layer. Per-graph pooled sums are AllReduced; the tiny MLP
head is computed redundantly on every core.

Hot path per (layer, 128-target block):
  dma_gather pulls all the block's source rows (one instruction per table
  half; int16 indices limit a gather table to 32768 rows, so tables are
  addressed as two base-offset halves);
  per 128-edge chunk, one fused DVE tensor_scalar builds the bf16 selection
  matrix S[e,t] = w'[e] * (tl[e]==t); PE accumulates aggT[f,t] += xr.T @ S in
  PSUM; PE transform hT[o,t] = W'.T @ aggT; scalar engine does
  relu(+bias')+BN-shift; Pool engine adds the residual; PE transposes back to
  row-major [t,o] for the bf16 activation table / pooling.
"""
import math
import os
import sys

import numpy as np

sys.path.insert(0, "/opt/trn_rl_repo")

N_NODES = 50000
N_EDGES = 800000
IN_DIM = 128
HID = 256
OUT_DIM = 1
N_GRAPHS = 512
BN_EPS = 1e-5
NCORES = 8
P = 128
SHARD = N_NODES // NCORES            # 6250
NBLK = (SHARD + P - 1) // P          # 49
PADN = NBLK * P                      # 6272 rows per core incl pad
SPLIT_BLK = 25                       # blocks per first AllGather half
H0 = SPLIT_BLK * P                   # 3200
H1 = PADN - H0                       # 3072
XROWS = PADN * NCORES                # 50176 rows in allgathered tables
KS = 32768                           # int16 gather-index limit (L1 x table)
XA = NCORES * H0                     # 25600 rows in AG half0 table
XB = NCORES * H1                     # 24576 rows in AG half1 table


def _build_program(cA1, cB1, cA2, cB2):
    from concourse import bass, bacc, mybir, tile
    from concourse.masks import make_identity

    f32 = mybir.dt.float32
    bf16 = mybir.dt.bfloat16
    i16 = mybir.dt.int16
    AF = mybir.ActivationFunctionType
    OP = mybir.AluOpType

    TOT1 = sum(cA1) + sum(cB1)
    TOT2 = sum(cA2) + sum(cB2)

    nc = bacc.Bacc("TRN2", target_bir_lowering=False, debug=False,
                   num_devices=NCORES)

    xbf = nc.declare_dram_parameter("xbf", [N_NODES, IN_DIM], bf16, isOutput=False)
    idx1 = nc.declare_dram_parameter("idx1", [P, 8 * TOT1], i16, isOutput=False)
    meta1 = nc.declare_dram_parameter("meta1", [P, 2 * TOT1], f32, isOutput=False)
    idx2 = nc.declare_dram_parameter("idx2", [P, 8 * TOT2], i16, isOutput=False)
    meta2 = nc.declare_dram_parameter("meta2", [P, 2 * TOT2], f32, isOutput=False)
    bcolp = nc.declare_dram_parameter("bcolp", [P, NBLK], f32, isOutput=False)
    w1p = nc.declare_dram_parameter("w1p", [IN_DIM, HID], bf16, isOutput=False)
    w2p = nc.declare_dram_parameter("w2p", [HID, HID], bf16, isOutput=False)
    w3p = nc.declare_dram_parameter("w3p", [HID, HID], bf16, isOutput=False)
    bias = nc.declare_dram_parameter("bias", [P, 6], f32, isOutput=False)
    tsh = nc.declare_dram_parameter("tsh", [P, 6], f32, isOutput=False)
    lw1 = nc.declare_dram_parameter("lw1", [HID, HID], f32, isOutput=False)
    lb1c = nc.declare_dram_parameter("lb1c", [P, 2], f32, isOutput=False)
    lw2 = nc.declare_dram_parameter("lw2", [P, 2], f32, isOutput=False)
    lb2c = nc.declare_dram_parameter("lb2c", [1, 1], f32, isOutput=False)
    icnt = nc.declare_dram_parameter("icnt", [P, N_GRAPHS], f32, isOutput=False)
    out = nc.declare_dram_parameter("out", [1, N_GRAPHS], f32, isOutput=True)
    dbg_x1 = os.environ.get("GNN_DBG_X1")
    if dbg_x1:
        outx1a = nc.declare_dram_parameter("outx1a", [XA, HID], bf16, isOutput=True)
        outx1b = nc.declare_dram_parameter("outx1b", [XB, HID], bf16, isOutput=True)
        outx2a = nc.declare_dram_parameter("outx2a", [XA, HID], bf16, isOutput=True)
        outx2b = nc.declare_dram_parameter("outx2b", [XB, HID], bf16, isOutput=True)

    with tile.TileContext(nc) as tc:
        with tc.tile_pool(name="const", bufs=1) as cpool, \
             tc.tile_pool(name="rows", bufs=3) as rpool, \
             tc.tile_pool(name="smat", bufs=8) as spool, \
             tc.tile_pool(name="work", bufs=6) as wpool, \
             tc.tile_pool(name="tail", bufs=1) as tpool, \
             tc.tile_pool(name="resid", bufs=1) as residp, \
             tc.tile_pool(name="hrow", bufs=3) as hpool, \
             tc.tile_pool(name="psum", bufs=2, space="PSUM") as ppool, \
             tc.tile_pool(name="psumt", bufs=2, space="PSUM") as ppoolt, \
             tc.tile_pool(name="psump", bufs=1, space="PSUM") as ppoolp, \
             tc.tile_pool(name="dram", bufs=8, space="DRAM") as dpool:

            iota_i = cpool.tile([P, P], mybir.dt.int32, tag="ioi")
            nc.gpsimd.iota(iota_i[:], pattern=[[1, P]], base=0, channel_multiplier=0)
            iota_b = cpool.tile([P, P], bf16, tag="iob")
            nc.vector.tensor_copy(iota_b[:], iota_i[:])
            iota5_i = cpool.tile([P, N_GRAPHS], mybir.dt.int32, tag="io5i")
            nc.gpsimd.iota(iota5_i[:], pattern=[[1, N_GRAPHS]], base=0, channel_multiplier=0)
            iota5_f = cpool.tile([P, N_GRAPHS], f32, tag="io5f")
            nc.vector.tensor_copy(iota5_f[:], iota5_i[:])
            ident = cpool.tile([P, P], f32, tag="ident")
            make_identity(nc, ident[:])

            bias_t = cpool.tile([P, 6], f32, tag="bias")
            nc.sync.dma_start(out=bias_t[:], in_=bias[:, :])
            tsh_t = cpool.tile([P, 6], f32, tag="tsh")
            nc.sync.dma_start(out=tsh_t[:], in_=tsh[:, :])
            bcol_t = cpool.tile([P, NBLK], f32, tag="bcol")
            nc.sync.dma_start(out=bcol_t[:], in_=bcolp[:, :])

            w1_t = cpool.tile([IN_DIM, HID], bf16, tag="w1")
            nc.sync.dma_start(out=w1_t[:], in_=w1p[:, :])
            w2_t = [cpool.tile([P, HID], bf16, tag=f"w2_{k}", name=f"w2_{k}") for k in range(2)]
            w3_t = [cpool.tile([P, HID], bf16, tag=f"w3_{k}", name=f"w3_{k}") for k in range(2)]
            for k in range(2):
                nc.sync.dma_start(out=w2_t[k][:], in_=w2p[k * P:(k + 1) * P, :])
                nc.sync.dma_start(out=w3_t[k][:], in_=w3p[k * P:(k + 1) * P, :])

            idx1_t = cpool.tile([P, 8 * TOT1], i16, tag="idx1")
            nc.sync.dma_start(out=idx1_t[:], in_=idx1[:, :])
            meta1_t = cpool.tile([P, 2 * TOT1], f32, tag="meta1")
            nc.sync.dma_start(out=meta1_t[:], in_=meta1[:, :])
            idx2_t = cpool.tile([P, 8 * TOT2], i16, tag="idx2")
            nc.sync.dma_start(out=idx2_t[:], in_=idx2[:, :])
            meta2_t = cpool.tile([P, 2 * TOT2], f32, tag="meta2")
            nc.sync.dma_start(out=meta2_t[:], in_=meta2[:, :])

            hloc1 = dpool.tile([PADN, HID], bf16, tag="hloc1")
            hloc2 = dpool.tile([PADN, HID], bf16, tag="hloc2")
            xn1a = dpool.tile([XA, HID], bf16, tag="xn1a")
            xn1b = dpool.tile([XB, HID], bf16, tag="xn1b")
            xn2a = dpool.tile([XA, HID], bf16, tag="xn2a")
            xn2b = dpool.tile([XB, HID], bf16, tag="xn2b")
            prdram = dpool.tile([HID, N_GRAPHS], f32, tag="prd")
            ardram = dpool.tile([HID, N_GRAPHS], f32, tag="ard")

            resid = [[residp.tile([P, P], f32, tag=f"r{b}h{h}", name=f"r{b}h{h}") for h in range(2)]
                     for b in range(NBLK)]

            pooled_ps = [ppoolp.tile([P, N_GRAPHS], f32, tag=f"pool{h}", name=f"pool{h}")
                         for h in range(2)]

            groups = [list(range(NCORES))]

            def allgather_half(hloc, xna, xnb, half):
                if half == 0:
                    nc.gpsimd.collective_compute(
                        "AllGather", OP.bypass, replica_groups=groups,
                        ins=[hloc[0:H0, :]], outs=[xna[:, :]])
                else:
                    nc.gpsimd.collective_compute(
                        "AllGather", OP.bypass, replica_groups=groups,
                        ins=[hloc[H0:PADN, :]], outs=[xnb[:, :]])

            def layer(li, tabA, tabB, fdim, idx_t, meta_t, cAs, cBs, wtiles,
                      bc0, hloc, mid_cb=None):
                nf = fdim // P
                io = 0
                mo = 0
                nblk_dbg = int(os.environ.get("GNN_DBG_NBLK", str(NBLK)))
                for b in range(NBLK):
                    if b >= nblk_dbg:
                        break
                    ca, cb = cAs[b], cBs[b]
                    C = ca + cb
                    xr = rpool.tile([P, C, fdim], bf16, tag="xr")
                    # dma_gather tops out at 1024 indices (8 chunks) per call
                    for tab, c0, c1 in ((tabA, 0, ca), (tabB, ca, C)):
                        for s in range(c0, c1, 8):
                            e = min(s + 8, c1)
                            nc.gpsimd.dma_gather(
                                xr[:, s:e, :], tab,
                                idx_t[:, io + 8 * s:io + 8 * e],
                                (e - s) * P, (e - s) * P, fdim)
                    io += 8 * C

                    aggT = [ppool.tile([P, P], f32, tag=f"agg{k}", name=f"aggps{k}")
                            for k in range(nf)]
                    for j in range(C):
                        smat = spool.tile([P, P], bf16, tag="smat")
                        nc.vector.tensor_scalar(
                            out=smat[:], in0=iota_b[:],
                            scalar1=meta_t[:, mo + j:mo + j + 1],
                            scalar2=meta_t[:, mo + C + j:mo + C + j + 1],
                            op0=OP.is_equal, op1=OP.mult)
                        for k in range(nf):
                            nc.tensor.matmul(
                                aggT[k][:], lhsT=xr[:, j, k * P:(k + 1) * P],
                                rhs=smat[:], start=(j == 0), stop=(j == C - 1))
                    mo += 2 * C

                    aggs = [wpool.tile([P, P], bf16, tag=f"aggs{k}", name=f"aggs{k}")
                            for k in range(nf)]
                    for k in range(nf):
                        nc.scalar.activation(aggs[k][:], aggT[k][:], AF.Copy)

                    hrow = hpool.tile([P, HID], bf16, tag="hrow")
                    for h in range(2):
                        hT_ps = ppoolt.tile([P, P], f32, tag="tmp")
                        for k in range(nf):
                            nc.tensor.matmul(
                                hT_ps[:], lhsT=wtiles[k][:, h * P:(h + 1) * P],
                                rhs=aggs[k][:], start=(k == 0), stop=(k == nf - 1))
                        hTs = wpool.tile([P, P], f32, tag=f"hTs{h}")
                        nc.scalar.activation(hTs[:], hT_ps[:], AF.Relu,
                                             bias=bias_t[:, bc0 + h:bc0 + h + 1])
                        if li == 0:
                            nc.scalar.activation(resid[b][h][:], hTs[:], AF.Identity,
                                                 bias=tsh_t[:, bc0 + h:bc0 + h + 1])
                        else:
                            u = wpool.tile([P, P], f32, tag=f"u{h}")
                            nc.scalar.activation(u[:], hTs[:], AF.Identity,
                                                 bias=tsh_t[:, bc0 + h:bc0 + h + 1])
                            nc.gpsimd.tensor_tensor(
                                out=resid[b][h][:], in0=resid[b][h][:], in1=u[:],
                                op=OP.add)
                        tp_ps = ppoolt.tile([P, P], f32, tag="tmp")
                        nc.tensor.transpose(tp_ps[:], resid[b][h][:], ident[:])
                        nc.scalar.activation(hrow[:, h * P:(h + 1) * P], tp_ps[:], AF.Copy)

                    if hloc is not None:
                        nc.sync.dma_start(out=hloc[b * P:(b + 1) * P, :], in_=hrow[:])
                        if b == SPLIT_BLK - 1 and mid_cb is not None:
                            mid_cb()
                    else:
                        mblk = spool.tile([P, N_GRAPHS], bf16, tag="mblk")
                        nc.vector.tensor_scalar(
                            out=mblk[:], in0=iota5_f[:],
                            scalar1=bcol_t[:, b:b + 1], scalar2=None,
                            op0=OP.is_equal)
                        for h in range(2):
                            nc.tensor.matmul(
                                pooled_ps[h][:], lhsT=hrow[:, h * P:(h + 1) * P],
                                rhs=mblk[:], start=(b == 0), stop=(b == NBLK - 1))

            dbg_stop = os.environ.get("GNN_DBG_STOP", "full")
            layer(0, xbf[0:KS, :], xbf[KS:N_NODES, :], IN_DIM, idx1_t, meta1_t,
                  cA1, cB1, [w1_t], 0, hloc1,
                  mid_cb=(lambda: allgather_half(hloc1, xn1a, xn1b, 0))
                  if dbg_stop != "l1" else None)
            if dbg_stop != "l1":
                allgather_half(hloc1, xn1a, xn1b, 1)
            if dbg_stop in ("l2", "ag2", "full"):
                layer(1, xn1a[:, :], xn1b[:, :], HID, idx2_t, meta2_t,
                      cA2, cB2, w2_t, 2, hloc2,
                      mid_cb=(lambda: allgather_half(hloc2, xn2a, xn2b, 0))
                      if dbg_stop != "l2" else None)
            if dbg_stop in ("ag2", "full"):
                allgather_half(hloc2, xn2a, xn2b, 1)
            if dbg_stop == "full":
                layer(2, xn2a[:, :], xn2b[:, :], HID, idx2_t, meta2_t,
                      cA2, cB2, w3_t, 4, None)
            else:
                # dummy pooled so the tail still builds
                dummy = wpool.tile([P, N_GRAPHS], bf16, tag="dummy")
                nc.vector.tensor_copy(dummy[:], iota5_f[:])
                for h in range(2):
                    nc.tensor.matmul(pooled_ps[h][:], lhsT=dummy[:, 0:P],
                                     rhs=dummy[:], start=True, stop=True)

            # pooled partial sums -> DRAM -> AllReduce
            icnt_t = cpool.tile([P, N_GRAPHS], f32, tag="icnt")
            nc.sync.dma_start(out=icnt_t[:], in_=icnt[:, :])
            for h in range(2):
                ps = tpool.tile([P, N_GRAPHS], f32, tag=f"poolsb{h}")
                nc.scalar.activation(ps[:], pooled_ps[h][:], AF.Copy)
                nc.sync.dma_start(out=prdram[h * P:(h + 1) * P, :], in_=ps[:])
            nc.gpsimd.collective_compute(
                "AllReduce", OP.add, replica_groups=groups,
                ins=[prdram[:, :]], outs=[ardram[:, :]])

            # head: h1T[o,g] = relu(lw1.T @ (pooledT*icnt) + lb1); out = lw2.T @ h1T + lb2
            lw1_t = [cpool.tile([P, HID], f32, tag=f"lw1_{k}", name=f"lw1_{k}") for k in range(2)]
            lw2_t = cpool.tile([P, 2], f32, tag="lw2")
            lb1_t = cpool.tile([P, 2], f32, tag="lb1")
            lb2_t = cpool.tile([1, 1], f32, tag="lb2")
            for k in range(2):
                nc.sync.dma_start(out=lw1_t[k][:], in_=lw1[k * P:(k + 1) * P, :])
            nc.sync.dma_start(out=lw2_t[:], in_=lw2[:, :])
            nc.sync.dma_start(out=lb1_t[:], in_=lb1c[:, :])
            nc.sync.dma_start(out=lb2_t[:], in_=lb2c[:, :])

            par = []
            for k in range(2):
                pk = tpool.tile([P, N_GRAPHS], f32, tag=f"par{k}")
                nc.sync.dma_start(out=pk[:], in_=ardram[k * P:(k + 1) * P, :])
                pks = tpool.tile([P, N_GRAPHS], f32, tag=f"pars{k}")
                nc.vector.tensor_tensor(out=pks[:], in0=pk[:], in1=icnt_t[:], op=OP.mult)
                par.append(pks)
            h1s = []
            for h in range(2):
                h1_ps = ppool.tile([P, N_GRAPHS], f32, tag="agg0")
                for k in range(2):
                    nc.tensor.matmul(h1_ps[:], lhsT=lw1_t[k][:, h * P:(h + 1) * P],
                                     rhs=par[k][:], start=(k == 0), stop=(k == 1))
                h1sb = tpool.tile([P, N_GRAPHS], f32, tag=f"h1s{h}")
                nc.scalar.activation(h1sb[:], h1_ps[:], AF.Relu,
                                     bias=lb1_t[:, h:h + 1])
                h1s.append(h1sb)
            out_ps = ppool.tile([1, N_GRAPHS], f32, tag="agg1")
            for h in range(2):
                nc.tensor.matmul(out_ps[:], lhsT=lw2_t[:, h:h + 1],
                                 rhs=h1s[h][:], start=(h == 0), stop=(h == 1))
            out_sb = tpool.tile([1, N_GRAPHS], f32, tag="outs")
            nc.vector.tensor_scalar(out=out_sb[:], in0=out_ps[:],
                                    scalar1=lb2_t[0:1, 0:1], scalar2=None, op0=OP.add)
            nc.sync.dma_start(out=out[:, :], in_=out_sb[:])
            if dbg_x1:
                nc.sync.dma_start(out=outx1a[:, :], in_=xn1a[:, :])
                nc.sync.dma_start(out=outx1b[:, :], in_=xn1b[:, :])
                nc.sync.dma_start(out=outx2a[:, :], in_=xn2a[:, :])
                nc.sync.dma_start(out=outx2b[:, :], in_=xn2b[:, :])

    nc.compile()
    return nc


def _wrap16(flat):
    """int16 index list (len % 128 == 0) -> [128, len/16] wrap-16 layout,
    replicated across the 8 gpsimd core groups."""
    cols = len(flat) // 16
    return np.tile(flat.reshape(cols, 16).T, (8, 1)).astype(np.int16)


def _preprocess(edge_index, batch):
    src = np.asarray(edge_index[0], dtype=np.int64)
    tgt = np.asarray(edge_index[1], dtype=np.int64)
    batch = np.asarray(batch, dtype=np.int64)

    deg = np.bincount(tgt, minlength=N_NODES).astype(np.float64) + 1.0
    dinv = 1.0 / np.sqrt(deg)

    allsrc = np.concatenate([src, np.arange(N_NODES, dtype=np.int64)])
    alltgt = np.concatenate([tgt, np.arange(N_NODES, dtype=np.int64)])
    allw = (dinv[allsrc] * dinv[alltgt]).astype(np.float32)

    order = np.argsort(alltgt, kind="stable")
    allsrc, alltgt, allw = allsrc[order], alltgt[order], allw[order]

    coreid = alltgt // SHARD
    locid = alltgt - coreid * SHARD
    blkkey = coreid * NBLK + locid // P
    tloc = (locid % P).astype(np.float32)

    # remapped row ids in the split-allgathered activation table
    cs = allsrc // SHARD
    rs = allsrc - cs * SHARD
    rid = np.where(rs < H0, cs * H0 + rs, NCORES * H0 + cs * H1 + (rs - H0))

    blk_start = np.zeros(NBLK * NCORES + 1, dtype=np.int64)
    np.cumsum(np.bincount(blkkey, minlength=NBLK * NCORES), out=blk_start[1:])

    # per (core, block) A/B edge counts for both index spaces
    isB1 = allsrc >= KS
    isB2 = rid >= NCORES * H0

    def chunk_counts(isB):
        nA = np.zeros((NCORES, NBLK), np.int64)
        nB = np.zeros((NCORES, NBLK), np.int64)
        for c in range(NCORES):
            for b in range(NBLK):
                g = c * NBLK + b
                m = isB[blk_start[g]:blk_start[g + 1]]
                nB[c, b] = int(m.sum())
                nA[c, b] = len(m) - nB[c, b]
        cA = [int(math.ceil(nA[:, b].max() / P)) for b in range(NBLK)]
        cB = [int(math.ceil(nB[:, b].max() / P)) for b in range(NBLK)]
        return cA, cB

    cA1, cB1 = chunk_counts(isB1)
    cA2, cB2 = chunk_counts(isB2)
    TOT1 = sum(cA1) + sum(cB1)
    TOT2 = sum(cA2) + sum(cB2)

    per_core = []
    for c in range(NCORES):
        idx1 = np.zeros((P, 8 * TOT1), np.int16)
        meta1 = np.zeros((P, 2 * TOT1), np.float32)
        idx2 = np.zeros((P, 8 * TOT2), np.int16)
        meta2 = np.zeros((P, 2 * TOT2), np.float32)
        io1 = mo1 = io2 = mo2 = 0
        for b in range(NBLK):
            g = c * NBLK + b
            lo, hi = blk_start[g], blk_start[g + 1]
            s = allsrc[lo:hi]
            r = rid[lo:hi]
            t = tloc[lo:hi]
            w = allw[lo:hi]

            for (ids, isb, cA, cB, KSo, idxa, metaa, io, mo) in (
                    (s, isB1[lo:hi], cA1[b], cB1[b], KS, idx1, meta1, io1, mo1),
                    (r, isB2[lo:hi], cA2[b], cB2[b], NCORES * H0, idx2, meta2, io2, mo2)):
                C = cA + cB
                mA, mB = ~isb, isb
                na, nb = int(mA.sum()), int(mB.sum())
                ia = np.zeros(cA * P, np.int64)
                ib = np.zeros(cB * P, np.int64)
                ia[:na] = ids[mA]
                ib[:nb] = ids[mB] - KSo
                tt = np.zeros(C * P, np.float32)
                ww = np.zeros(C * P, np.float32)
                tt[:na] = t[mA]
                tt[cA * P:cA * P + nb] = t[mB]
                ww[:na] = w[mA]
                ww[cA * P:cA * P + nb] = w[mB]
                if cA:
                    idxa[:, io:io + 8 * cA] = _wrap16(ia)
                if cB:
                    idxa[:, io + 8 * cA:io + 8 * C] = _wrap16(ib)
                metaa[:, mo:mo + C] = tt.reshape(C, P).T
                metaa[:, mo + C:mo + 2 * C] = ww.reshape(C, P).T

            io1 += 8 * (cA1[b] + cB1[b])
            mo1 += 2 * (cA1[b] + cB1[b])
            io2 += 8 * (cA2[b] + cB2[b])
            mo2 += 2 * (cA2[b] + cB2[b])

        # batch column for pooling (pad rows -> -1)
        bvals = batch[c * SHARD:(c + 1) * SHARD].astype(np.float32)
        bpad = np.pad(bvals, (0, PADN - SHARD), constant_values=-1.0)
        bcol = bpad.reshape(NBLK, P).T.copy()  # [P, NBLK]
        per_core.append(dict(idx1=idx1, meta1=meta1, idx2=idx2, meta2=meta2,
                             bcolp=bcol))
    return per_core, cA1, cB1, cA2, cB2


def kernel(**inputs):
    import ml_dtypes
    from concourse.bass_utils import run_bass_kernel_spmd

    x = np.asarray(inputs["x"], dtype=np.float32)
    edge_index = np.asarray(inputs["edge_index"])
    batch = np.asarray(inputs["batch"])

    per_core, cA1, cB1, cA2, cB2 = _preprocess(edge_index, batch)

    def g(k):
        return np.asarray(inputs[k], dtype=np.float32)

    params = {}
    params["xbf"] = x.astype(ml_dtypes.bfloat16)
    Ws = [g("W1"), g("W2"), g("W3")]
    bs = [g("b1"), g("b2"), g("b3")]
    bias = np.zeros((P, 6), np.float32)
    tshv = np.zeros((P, 6), np.float32)
    wp = []
    for i in range(3):
        gam, be, m, v = g(f"g{i+1}"), g(f"be{i+1}"), g(f"m{i+1}"), g(f"v{i+1}")
        s = gam / np.sqrt(v + BN_EPS)
        assert (s > 0).all(), "BN scale must be positive for relu folding"
        wp.append((Ws[i] * s[None, :]).astype(ml_dtypes.bfloat16))
        bp = (bs[i] * s).astype(np.float32)
        tv = (be - m * s).astype(np.float32)
        bias[:, 2 * i] = bp[:P]
        bias[:, 2 * i + 1] = bp[P:]
        tshv[:, 2 * i] = tv[:P]
        tshv[:, 2 * i + 1] = tv[P:]
    params["w1p"], params["w2p"], params["w3p"] = wp
    params["bias"] = bias
    params["tsh"] = tshv
    params["lw1"] = g("lw1")
    lb1 = g("lb1")
    lb1c = np.zeros((P, 2), np.float32)
    lb1c[:, 0] = lb1[:P]
    lb1c[:, 1] = lb1[P:]
    params["lb1c"] = lb1c
    lw2v = g("lw2").reshape(HID)
    params["lw2"] = np.stack([lw2v[:P], lw2v[P:]], axis=1).copy()
    params["lb2c"] = g("lb2").reshape(1, 1).astype(np.float32)
    cnt = np.bincount(np.asarray(batch, dtype=np.int64), minlength=N_GRAPHS)
    icnt = (1.0 / np.maximum(cnt, 1)).astype(np.float32)
    params["icnt"] = np.tile(icnt[None, :], (P, 1))

    nc = _build_program(cA1, cB1, cA2, cB2)

    in_maps = []
    for c in range(NCORES):
        m = dict(params)
        m.update(per_core[c])
        in_maps.append(m)

    res = run_bass_kernel_spmd(nc, in_maps, list(range(NCORES)),
                               trace=bool(os.environ.get("GNN_TRACE")))
    if os.environ.get("GNN_TRACE"):
        print("HW exec time:", res.exec_time_ns, "ns")
    global _last_results
    _last_results = res
    o = res.results[0]["out"]
    return np.asarray(o, dtype=np.float32).reshape(N_GRAPHS, OUT_DIM)


# revision 15
# speedup vs baseline: 1.8809x; 1.8809x over previous
"""GCN (3x GCNConv + BN + residual, mean-pool, MLP head) on 8 trn2 NeuronCores.

Sharding: nodes split contiguously across 8 cores (6250 each); each core owns
the edges whose TARGET lands in its shard (plus self-loops). Per layer, each
core aggregates source features over its incident edges (GCN normalization is
linear, so aggregate-then-transform), applies the folded linear+BN epilogue,
and the bf16 activations are AllGathered (split in two halves so the first
half overlaps the tail of the layer) so every core can gather arbitrary
source rows next layer. Per-graph pooled sums are AllReduced; the tiny MLP
head is computed redundantly on every core.

Hot path per (layer, 128-target block):
  dma_gather pulls all the block's source rows (one instruction per table
  half; int16 indices limit a gather table to 32768 rows, so tables are
  addressed as two base-offset halves);
  per 128-edge chunk, one fused DVE tensor_scalar builds the bf16 selection
  matrix S[e,t] = w'[e] * (tl[e]==t); PE accumulates aggT[f,t] += xr.T @ S in
  PSUM; PE transform hT[o,t] = W'.T @ aggT; scalar engine does
  relu(+bias')+BN-shift; Pool engine adds the residual; PE transposes back to
  row-major [t,o] for the bf16 activation table / pooling.
"""
import math
import os
import sys

import numpy as np

sys.path.insert(0, "/opt/trn_rl_repo")

N_NODES = 50000
N_EDGES = 800000
IN_DIM = 128
HID = 256
OUT_DIM = 1
N_GRAPHS = 512
BN_EPS = 1e-5
NCORES = 8
P = 128
SHARD = N_NODES // NCORES            # 6250
NBLK = (SHARD + P - 1) // P          # 49
PADN = NBLK * P                      # 6272 rows per core incl pad
SPLIT_BLK = 25                       # blocks per first AllGather half
H0 = SPLIT_BLK * P                   # 3200
H1 = PADN - H0                       # 3072
XROWS = PADN * NCORES                # 50176 rows in allgathered tables
KS = 32768                           # int16 gather-index limit (L1 x table)
XA = NCORES * H0                     # 25600 rows in AG half0 table
XB = NCORES * H1                     # 24576 rows in AG half1 table


def _build_program(cA1, cB1, cA2, cB2):
    from concourse import bass, bacc, mybir, tile
    from concourse.masks import make_identity

    f32 = mybir.dt.float32
    bf16 = mybir.dt.bfloat16
    i16 = mybir.dt.int16
    AF = mybir.ActivationFunctionType
    OP = mybir.AluOpType

    TOT1 = sum(cA1) + sum(cB1)
    TOT2 = sum(cA2) + sum(cB2)

    nc = bacc.Bacc("TRN2", target_bir_lowering=False, debug=False,
                   num_devices=NCORES)

    xbf = nc.declare_dram_parameter("xbf", [N_NODES, IN_DIM], bf16, isOutput=False)
    idx1 = nc.declare_dram_parameter("idx1", [P, 8 * TOT1], i16, isOutput=False)
    meta1 = nc.declare_dram_parameter("meta1", [P, 2 * TOT1], f32, isOutput=False)
    idx2 = nc.declare_dram_parameter("idx2", [P, 8 * TOT2], i16, isOutput=False)
    meta2 = nc.declare_dram_parameter("meta2", [P, 2 * TOT2], f32, isOutput=False)
    bcolp = nc.declare_dram_parameter("bcolp", [P, NBLK], f32, isOutput=False)
    w1p = nc.declare_dram_parameter("w1p", [IN_DIM, HID], bf16, isOutput=False)
    w2p = nc.declare_dram_parameter("w2p", [HID, HID], bf16, isOutput=False)
    w3p = nc.declare_dram_parameter("w3p", [HID, HID], bf16, isOutput=False)
    bias = nc.declare_dram_parameter("bias", [P, 6], f32, isOutput=False)
    tsh = nc.declare_dram_parameter("tsh", [P, 6], f32, isOutput=False)
    lw1 = nc.declare_dram_parameter("lw1", [HID, HID], f32, isOutput=False)
    lb1c = nc.declare_dram_parameter("lb1c", [P, 2], f32, isOutput=False)
    lw2 = nc.declare_dram_parameter("lw2", [P, 2], f32, isOutput=False)
    lb2c = nc.declare_dram_parameter("lb2c", [1, 1], f32, isOutput=False)
    icnt = nc.declare_dram_parameter("icnt", [P, N_GRAPHS], f32, isOutput=False)
    out = nc.declare_dram_parameter("out", [1, N_GRAPHS], f32, isOutput=True)
    dbg_x1 = os.environ.get("GNN_DBG_X1")
    if dbg_x1:
        outx1a = nc.declare_dram_parameter("outx1a", [XA, HID], bf16, isOutput=True)
        outx1b = nc.declare_dram_parameter("outx1b", [XB, HID], bf16, isOutput=True)
        outx2a = nc.declare_dram_parameter("outx2a", [XA, HID], bf16, isOutput=True)
        outx2b = nc.declare_dram_parameter("outx2b", [XB, HID], bf16, isOutput=True)

    with tile.TileContext(nc) as tc:
        with tc.tile_pool(name="const", bufs=1) as cpool, \
             tc.tile_pool(name="rows", bufs=3) as rpool, \
             tc.tile_pool(name="smat", bufs=8) as spool, \
             tc.tile_pool(name="work", bufs=6) as wpool, \
             tc.tile_pool(name="tail", bufs=1) as tpool, \
             tc.tile_pool(name="resid", bufs=1) as residp, \
             tc.tile_pool(name="hrow", bufs=3) as hpool, \
             tc.tile_pool(name="psum", bufs=2, space="PSUM") as ppool, \
             tc.tile_pool(name="psumt", bufs=2, space="PSUM") as ppoolt, \
             tc.tile_pool(name="psump", bufs=1, space="PSUM") as ppoolp, \
             tc.tile_pool(name="dram", bufs=8, space="DRAM") as dpool:

            iota_i = cpool.tile([P, P], mybir.dt.int32, tag="ioi")
            nc.gpsimd.iota(iota_i[:], pattern=[[1, P]], base=0, channel_multiplier=0)
            iota_b = cpool.tile([P, P], bf16, tag="iob")
            nc.vector.tensor_copy(iota_b[:], iota_i[:])
            iota5_i = cpool.tile([P, N_GRAPHS], mybir.dt.int32, tag="io5i")
            nc.gpsimd.iota(iota5_i[:], pattern=[[1, N_GRAPHS]], base=0, channel_multiplier=0)
            iota5_f = cpool.tile([P, N_GRAPHS], f32, tag="io5f")
            nc.vector.tensor_copy(iota5_f[:], iota5_i[:])
            ident = cpool.tile([P, P], f32, tag="ident")
            make_identity(nc, ident[:])

            bias_t = cpool.tile([P, 6], f32, tag="bias")
            nc.sync.dma_start(out=bias_t[:], in_=bias[:, :])
            tsh_t = cpool.tile([P, 6], f32, tag="tsh")
            nc.sync.dma_start(out=tsh_t[:], in_=tsh[:, :])
            bcol_t = cpool.tile([P, NBLK], f32, tag="bcol")
            nc.sync.dma_start(out=bcol_t[:], in_=bcolp[:, :])

            w1_t = cpool.tile([IN_DIM, HID], bf16, tag="w1")
            nc.sync.dma_start(out=w1_t[:], in_=w1p[:, :])
            w2_t = [cpool.tile([P, HID], bf16, tag=f"w2_{k}", name=f"w2_{k}") for k in range(2)]
            w3_t = [cpool.tile([P, HID], bf16, tag=f"w3_{k}", name=f"w3_{k}") for k in range(2)]
            for k in range(2):
                nc.sync.dma_start(out=w2_t[k][:], in_=w2p[k * P:(k + 1) * P, :])
                nc.sync.dma_start(out=w3_t[k][:], in_=w3p[k * P:(k + 1) * P, :])

            idx1_t = cpool.tile([P, 8 * TOT1], i16, tag="idx1")
            nc.sync.dma_start(out=idx1_t[:], in_=idx1[:, :])
            meta1_t = cpool.tile([P, 2 * TOT1], f32, tag="meta1")
            nc.sync.dma_start(out=meta1_t[:], in_=meta1[:, :])
            idx2_t = cpool.tile([P, 8 * TOT2], i16, tag="idx2")
            nc.sync.dma_start(out=idx2_t[:], in_=idx2[:, :])
            meta2_t = cpool.tile([P, 2 * TOT2], f32, tag="meta2")
            nc.sync.dma_start(out=meta2_t[:], in_=meta2[:, :])

            hloc1 = dpool.tile([PADN, HID], bf16, tag="hloc1")
            hloc2 = dpool.tile([PADN, HID], bf16, tag="hloc2")
            xn1a = dpool.tile([XA, HID], bf16, tag="xn1a")
            xn1b = dpool.tile([XB, HID], bf16, tag="xn1b")
            xn2a = dpool.tile([XA, HID], bf16, tag="xn2a")
            xn2b = dpool.tile([XB, HID], bf16, tag="xn2b")
            prdram = dpool.tile([HID, N_GRAPHS], f32, tag="prd")
            ardram = dpool.tile([HID, N_GRAPHS], f32, tag="ard")

            resid = [[residp.tile([P, P], f32, tag=f"r{b}h{h}", name=f"r{b}h{h}") for h in range(2)]
                     for b in range(NBLK)]

            pooled_ps = [ppoolp.tile([P, N_GRAPHS], f32, tag=f"pool{h}", name=f"pool{h}")
                         for h in range(2)]

            groups = [list(range(NCORES))]

            def allgather_half(hloc, xna, xnb, half):
                if half == 0:
                    nc.gpsimd.collective_compute(
                        "AllGather", OP.bypass, replica_groups=groups,
                        ins=[hloc[0:H0, :]], outs=[xna[:, :]])
                else:
                    nc.gpsimd.collective_compute(
                        "AllGather", OP.bypass, replica_groups=groups,
                        ins=[hloc[H0:PADN, :]], outs=[xnb[:, :]])

            def layer(li, tabA, tabB, fdim, idx_t, meta_t, cAs, cBs, wtiles,
                      bc0, hloc, mid_cb=None):
                nf = fdim // P
                io = 0
                mo = 0
                nblk_dbg = int(os.environ.get("GNN_DBG_NBLK", str(NBLK)))
                for b in range(NBLK):
                    if b >= nblk_dbg:
                        break
                    ca, cb = cAs[b], cBs[b]
                    C = ca + cb
                    xr = rpool.tile([P, C, fdim], bf16, tag="xr")
                    # dma_gather tops out at 1024 indices (8 chunks) per call
                    for tab, c0, c1 in ((tabA, 0, ca), (tabB, ca, C)):
                        for s in range(c0, c1, 8):
                            e = min(s + 8, c1)
                            nc.gpsimd.dma_gather(
                                xr[:, s:e, :], tab,
                                idx_t[:, io + 8 * s:io + 8 * e],
                                (e - s) * P, (e - s) * P, fdim)
                    io += 8 * C

                    aggT = [ppool.tile([P, P], f32, tag=f"agg{k}", name=f"aggps{k}")
                            for k in range(nf)]
                    for j in range(C):
                        smat = spool.tile([P, P], bf16, tag="smat")
                        nc.vector.tensor_scalar(
                            out=smat[:], in0=iota_b[:],
                            scalar1=meta_t[:, mo + j:mo + j + 1],
                            scalar2=meta_t[:, mo + C + j:mo + C + j + 1],
                            op0=OP.is_equal, op1=OP.mult)
                        for k in range(nf):
                            nc.tensor.matmul(
                                aggT[k][:], lhsT=xr[:, j, k * P:(k + 1) * P],
                                rhs=smat[:], start=(j == 0), stop=(j == C - 1))
                    mo += 2 * C

                    aggs = [wpool.tile([P, P], bf16, tag=f"aggs{k}", name=f"aggs{k}")
                            for k in range(nf)]
                    for k in range(nf):
                        nc.scalar.activation(aggs[k][:], aggT[k][:], AF.Copy)

                    hrow = hpool.tile([P, HID], bf16, tag="hrow")
                    for h in range(2):
                        hT_ps = ppoolt.tile([P, P], f32, tag="tmp")
                        for k in range(nf):
                            nc.tensor.matmul(
                                hT_ps[:], lhsT=wtiles[k][:, h * P:(h + 1) * P],
                                rhs=aggs[k][:], start=(k == 0), stop=(k == nf - 1))
                        hTs = wpool.tile([P, P], f32, tag=f"hTs{h}")
                        nc.scalar.activation(hTs[:], hT_ps[:], AF.Relu,
                                             bias=bias_t[:, bc0 + h:bc0 + h + 1])
                        if li == 0:
                            nc.scalar.activation(resid[b][h][:], hTs[:], AF.Identity,
                                                 bias=tsh_t[:, bc0 + h:bc0 + h + 1])
                        else:
                            u = wpool.tile([P, P], f32, tag=f"u{h}")
                            nc.scalar.activation(u[:], hTs[:], AF.Identity,
                                                 bias=tsh_t[:, bc0 + h:bc0 + h + 1])
                            nc.vector.tensor_tensor(
                                out=resid[b][h][:], in0=resid[b][h][:], in1=u[:],
                                op=OP.add)
                        tp_ps = ppoolt.tile([P, P], f32, tag="tmp")
                        nc.tensor.transpose(tp_ps[:], resid[b][h][:], ident[:])
                        nc.scalar.activation(hrow[:, h * P:(h + 1) * P], tp_ps[:], AF.Copy)

                    if hloc is not None:
                        nc.sync.dma_start(out=hloc[b * P:(b + 1) * P, :], in_=hrow[:])
                        if b == SPLIT_BLK - 1 and mid_cb is not None:
                            mid_cb()
                    else:
                        mblk = spool.tile([P, N_GRAPHS], bf16, tag="mblk")
                        nc.vector.tensor_scalar(
                            out=mblk[:], in0=iota5_f[:],
                            scalar1=bcol_t[:, b:b + 1], scalar2=None,
                            op0=OP.is_equal)
                        for h in range(2):
                            nc.tensor.matmul(
                                pooled_ps[h][:], lhsT=hrow[:, h * P:(h + 1) * P],
                                rhs=mblk[:], start=(b == 0), stop=(b == NBLK - 1))

            dbg_stop = os.environ.get("GNN_DBG_STOP", "full")
            layer(0, xbf[0:KS, :], xbf[KS:N_NODES, :], IN_DIM, idx1_t, meta1_t,
                  cA1, cB1, [w1_t], 0, hloc1,
                  mid_cb=(lambda: allgather_half(hloc1, xn1a, xn1b, 0))
                  if dbg_stop != "l1" else None)
            if dbg_stop != "l1":
                allgather_half(hloc1, xn1a, xn1b, 1)
            if dbg_stop in ("l2", "ag2", "full"):
                layer(1, xn1a[:, :], xn1b[:, :], HID, idx2_t, meta2_t,
                      cA2, cB2, w2_t, 2, hloc2,
                      mid_cb=(lambda: allgather_half(hloc2, xn2a, xn2b, 0))
                      if dbg_stop != "l2" else None)
            if dbg_stop in ("ag2", "full"):
                allgather_half(hloc2, xn2a, xn2b, 1)
            if dbg_stop == "full":
                layer(2, xn2a[:, :], xn2b[:, :], HID, idx2_t, meta2_t,
                      cA2, cB2, w3_t, 4, None)
            else:
                # dummy pooled so the tail still builds
                dummy = wpool.tile([P, N_GRAPHS], bf16, tag="dummy")
                nc.vector.tensor_copy(dummy[:], iota5_f[:])
                for h in range(2):
                    nc.tensor.matmul(pooled_ps[h][:], lhsT=dummy[:, 0:P],
                                     rhs=dummy[:], start=True, stop=True)

            # pooled partial sums -> DRAM -> AllReduce
            icnt_t = cpool.tile([P, N_GRAPHS], f32, tag="icnt")
            nc.sync.dma_start(out=icnt_t[:], in_=icnt[:, :])
            for h in range(2):
                ps = tpool.tile([P, N_GRAPHS], f32, tag=f"poolsb{h}")
                nc.scalar.activation(ps[:], pooled_ps[h][:], AF.Copy)
                nc.sync.dma_start(out=prdram[h * P:(h + 1) * P, :], in_=ps[:])
            nc.gpsimd.collective_compute(
                "AllReduce", OP.add, replica_groups=groups,
                ins=[prdram[:, :]], outs=[ardram[:, :]])

            # head: h1T[o,g] = relu(lw1.T @ (pooledT*icnt) + lb1); out = lw2.T @ h1T + lb2
            lw1_t = [cpool.tile([P, HID], f32, tag=f"lw1_{k}", name=f"lw1_{k}") for k in range(2)]
            lw2_t = cpool.tile([P, 2], f32, tag="lw2")
            lb1_t = cpool.tile([P, 2], f32, tag="lb1")
            lb2_t = cpool.tile([1, 1], f32, tag="lb2")
            for k in range(2):
                nc.sync.dma_start(out=lw1_t[k][:], in_=lw1[k * P:(k + 1) * P, :])
            nc.sync.dma_start(out=lw2_t[:], in_=lw2[:, :])
            nc.sync.dma_start(out=lb1_t[:], in_=lb1c[:, :])
            nc.sync.dma_start(out=lb2_t[:], in_=lb2c[:, :])

            par = []
            for k in range(2):
                pk = tpool.tile([P, N_GRAPHS], f32, tag=f"par{k}")
                nc.sync.dma_start(out=pk[:], in_=ardram[k * P:(k + 1) * P, :])
                pks = tpool.tile([P, N_GRAPHS], f32, tag=f"pars{k}")
                nc.vector.tensor_tensor(out=pks[:], in0=pk[:], in1=icnt_t[:], op=OP.mult)
                par.append(pks)
            h1s = []
            for h in range(2):
                h1_ps = ppool.tile([P, N_GRAPHS], f32, tag="agg0")
                for k in range(2):
                    nc.tensor.matmul(h1_ps[:], lhsT=lw1_t[k][:, h * P:(h + 1) * P],
                                     rhs=par[k][:], start=(k == 0), stop=(k == 1))
                h1sb = tpool.tile([P, N_GRAPHS], f32, tag=f"h1s{h}")
                nc.scalar.activation(h1sb[:], h1_ps[:], AF.Relu,
                                     bias=lb1_t[:, h:h + 1])
                h1s.append(h1sb)
            out_ps = ppool.tile([1, N_GRAPHS], f32, tag="agg1")
            for h in range(2):
                nc.tensor.matmul(out_ps[:], lhsT=lw2_t[:, h:h + 1],
                                 rhs=h1s[h][:], start=(h == 0), stop=(h == 1))
            out_sb = tpool.tile([1, N_GRAPHS], f32, tag="outs")
            nc.vector.tensor_scalar(out=out_sb[:], in0=out_ps[:],
                                    scalar1=lb2_t[0:1, 0:1], scalar2=None, op0=OP.add)
            nc.sync.dma_start(out=out[:, :], in_=out_sb[:])
            if dbg_x1:
                nc.sync.dma_start(out=outx1a[:, :], in_=xn1a[:, :])
                nc.sync.dma_start(out=outx1b[:, :], in_=xn1b[:, :])
                nc.sync.dma_start(out=outx2a[:, :], in_=xn2a[:, :])
                nc.sync.dma_start(out=outx2b[:, :], in_=xn2b[:, :])

    nc.compile()
    return nc


def _wrap16(flat):
    """int16 index list (len % 128 == 0) -> [128, len/16] wrap-16 layout,
    replicated across the 8 gpsimd core groups."""
    cols = len(flat) // 16
    return np.tile(flat.reshape(cols, 16).T, (8, 1)).astype(np.int16)


def _preprocess(edge_index, batch):
    src = np.asarray(edge_index[0], dtype=np.int64)
    tgt = np.asarray(edge_index[1], dtype=np.int64)
    batch = np.asarray(batch, dtype=np.int64)

    deg = np.bincount(tgt, minlength=N_NODES).astype(np.float64) + 1.0
    dinv = 1.0 / np.sqrt(deg)

    allsrc = np.concatenate([src, np.arange(N_NODES, dtype=np.int64)])
    alltgt = np.concatenate([tgt, np.arange(N_NODES, dtype=np.int64)])
    allw = (dinv[allsrc] * dinv[alltgt]).astype(np.float32)

    order = np.argsort(alltgt, kind="stable")
    allsrc, alltgt, allw = allsrc[order], alltgt[order], allw[order]

    coreid = alltgt // SHARD
    locid = alltgt - coreid * SHARD
    blkkey = coreid * NBLK + locid // P
    tloc = (locid % P).astype(np.float32)

    # remapped row ids in the split-allgathered activation table
    cs = allsrc // SHARD
    rs = allsrc - cs * SHARD
    rid = np.where(rs < H0, cs * H0 + rs, NCORES * H0 + cs * H1 + (rs - H0))

    blk_start = np.zeros(NBLK * NCORES + 1, dtype=np.int64)
    np.cumsum(np.bincount(blkkey, minlength=NBLK * NCORES), out=blk_start[1:])

    # per (core, block) A/B edge counts for both index spaces
    isB1 = allsrc >= KS
    isB2 = rid >= NCORES * H0

    def chunk_counts(isB):
        nA = np.zeros((NCORES, NBLK), np.int64)
        nB = np.zeros((NCORES, NBLK), np.int64)
        for c in range(NCORES):
            for b in range(NBLK):
                g = c * NBLK + b
                m = isB[blk_start[g]:blk_start[g + 1]]
                nB[c, b] = int(m.sum())
                nA[c, b] = len(m) - nB[c, b]
        cA = [int(math.ceil(nA[:, b].max() / P)) for b in range(NBLK)]
        cB = [int(math.ceil(nB[:, b].max() / P)) for b in range(NBLK)]
        return cA, cB

    cA1, cB1 = chunk_counts(isB1)
    cA2, cB2 = chunk_counts(isB2)
    TOT1 = sum(cA1) + sum(cB1)
    TOT2 = sum(cA2) + sum(cB2)

    per_core = []
    for c in range(NCORES):
        idx1 = np.zeros((P, 8 * TOT1), np.int16)
        meta1 = np.zeros((P, 2 * TOT1), np.float32)
        idx2 = np.zeros((P, 8 * TOT2), np.int16)
        meta2 = np.zeros((P, 2 * TOT2), np.float32)
        io1 = mo1 = io2 = mo2 = 0
        for b in range(NBLK):
            g = c * NBLK + b
            lo, hi = blk_start[g], blk_start[g + 1]
            s = allsrc[lo:hi]
            r = rid[lo:hi]
            t = tloc[lo:hi]
            w = allw[lo:hi]

            for (ids, isb, cA, cB, KSo, idxa, metaa, io, mo) in (
                    (s, isB1[lo:hi], cA1[b], cB1[b], KS, idx1, meta1, io1, mo1),
                    (r, isB2[lo:hi], cA2[b], cB2[b], NCORES * H0, idx2, meta2, io2, mo2)):
                C = cA + cB
                mA, mB = ~isb, isb
                na, nb = int(mA.sum()), int(mB.sum())
                ia = np.zeros(cA * P, np.int64)
                ib = np.zeros(cB * P, np.int64)
                ia[:na] = ids[mA]
                ib[:nb] = ids[mB] - KSo
                tt = np.zeros(C * P, np.float32)
                ww = np.zeros(C * P, np.float32)
                tt[:na] = t[mA]
                tt[cA * P:cA * P + nb] = t[mB]
                ww[:na] = w[mA]
                ww[cA * P:cA * P + nb] = w[mB]
                if cA:
                    idxa[:, io:io + 8 * cA] = _wrap16(ia)
                if cB:
                    idxa[:, io + 8 * cA:io + 8 * C] = _wrap16(ib)
                metaa[:, mo:mo + C] = tt.reshape(C, P).T
                metaa[:, mo + C:mo + 2 * C] = ww.reshape(C, P).T

            io1 += 8 * (cA1[b] + cB1[b])
            mo1 += 2 * (cA1[b] + cB1[b])
            io2 += 8 * (cA2[b] + cB2[b])
            mo2 += 2 * (cA2[b] + cB2[b])

        # batch column for pooling (pad rows -> -1)
        bvals = batch[c * SHARD:(c + 1) * SHARD].astype(np.float32)
        bpad = np.pad(bvals, (0, PADN - SHARD), constant_values=-1.0)
        bcol = bpad.reshape(NBLK, P).T.copy()  # [P, NBLK]
        per_core.append(dict(idx1=idx1, meta1=meta1, idx2=idx2, meta2=meta2,
                             bcolp=bcol))
    return per_core, cA1, cB1, cA2, cB2


def kernel(**inputs):
    import ml_dtypes
    from concourse.bass_utils import run_bass_kernel_spmd

    x = np.asarray(inputs["x"], dtype=np.float32)
    edge_index = np.asarray(inputs["edge_index"])
    batch = np.asarray(inputs["batch"])

    per_core, cA1, cB1, cA2, cB2 = _preprocess(edge_index, batch)

    def g(k):
        return np.asarray(inputs[k], dtype=np.float32)

    params = {}
    params["xbf"] = x.astype(ml_dtypes.bfloat16)
    Ws = [g("W1"), g("W2"), g("W3")]
    bs = [g("b1"), g("b2"), g("b3")]
    bias = np.zeros((P, 6), np.float32)
    tshv = np.zeros((P, 6), np.float32)
    wp = []
    for i in range(3):
        gam, be, m, v = g(f"g{i+1}"), g(f"be{i+1}"), g(f"m{i+1}"), g(f"v{i+1}")
        s = gam / np.sqrt(v + BN_EPS)
        assert (s > 0).all(), "BN scale must be positive for relu folding"
        wp.append((Ws[i] * s[None, :]).astype(ml_dtypes.bfloat16))
        bp = (bs[i] * s).astype(np.float32)
        tv = (be - m * s).astype(np.float32)
        bias[:, 2 * i] = bp[:P]
        bias[:, 2 * i + 1] = bp[P:]
        tshv[:, 2 * i] = tv[:P]
        tshv[:, 2 * i + 1] = tv[P:]
    params["w1p"], params["w2p"], params["w3p"] = wp
    params["bias"] = bias
    params["tsh"] = tshv
    params["lw1"] = g("lw1")
    lb1 = g("lb1")
    lb1c = np.zeros((P, 2), np.float32)
    lb1c[:, 0] = lb1[:P]
    lb1c[:, 1] = lb1[P:]
    params["lb1c"] = lb1c
    lw2v = g("lw2").reshape(HID)
    params["lw2"] = np.stack([lw2v[:P], lw2v[P:]], axis=1).copy()
    params["lb2c"] = g("lb2").reshape(1, 1).astype(np.float32)
    cnt = np.bincount(np.asarray(batch, dtype=np.int64), minlength=N_GRAPHS)
    icnt = (1.0 / np.maximum(cnt, 1)).astype(np.float32)
    params["icnt"] = np.tile(icnt[None, :], (P, 1))

    nc = _build_program(cA1, cB1, cA2, cB2)

    in_maps = []
    for c in range(NCORES):
        m = dict(params)
        m.update(per_core[c])
        in_maps.append(m)

    res = run_bass_kernel_spmd(nc, in_maps, list(range(NCORES)),
                               trace=bool(os.environ.get("GNN_TRACE")))
    if os.environ.get("GNN_TRACE"):
        print("HW exec time:", res.exec_time_ns, "ns")
    global _last_results
    _last_results = res
    o = res.results[0]["out"]
    return np.asarray(o, dtype=np.float32).reshape(N_GRAPHS, OUT_DIM)


# revision 16
# speedup vs baseline: 2.0069x; 1.0670x over previous
"""GCN (3x GCNConv + BN + residual, mean-pool, MLP head) on 8 trn2 NeuronCores.

Sharding: nodes split contiguously across 8 cores (6250 each); each core owns
the edges whose TARGET lands in its shard (plus self-loops). Per layer, each
core aggregates source features over its incident edges (GCN normalization is
linear, so aggregate-then-transform), applies the folded linear+BN epilogue,
and the bf16 activations are AllGathered (split in two halves so the first
half overlaps the tail of the layer) so every core can gather arbitrary
source rows next layer. Per-graph pooled sums are AllReduced; the tiny MLP
head is computed redundantly on every core.

Hot path per (layer, 128-target block):
  dma_gather pulls all the block's source rows (one instruction per table
  half; int16 indices limit a gather table to 32768 rows, so tables are
  addressed as two base-offset halves);
  per 128-edge chunk, one fused DVE tensor_scalar builds the bf16 selection
  matrix S[e,t] = w'[e] * (tl[e]==t); PE accumulates aggT[f,t] += xr.T @ S in
  PSUM; PE transform hT[o,t] = W'.T @ aggT; scalar engine does
  relu(+bias')+BN-shift; Pool engine adds the residual; PE transposes back to
  row-major [t,o] for the bf16 activation table / pooling.
"""
import math
import os
import sys

import numpy as np

sys.path.insert(0, "/opt/trn_rl_repo")

N_NODES = 50000
N_EDGES = 800000
IN_DIM = 128
HID = 256
OUT_DIM = 1
N_GRAPHS = 512
BN_EPS = 1e-5
NCORES = 8
P = 128
SHARD = N_NODES // NCORES            # 6250
NBLK = (SHARD + P - 1) // P          # 49
PADN = NBLK * P                      # 6272 rows per core incl pad
SPLIT_BLK = 25                       # blocks per first AllGather half
H0 = SPLIT_BLK * P                   # 3200
H1 = PADN - H0                       # 3072
XROWS = PADN * NCORES                # 50176 rows in allgathered tables
KS = 32768                           # int16 gather-index limit (L1 x table)
XA = NCORES * H0                     # 25600 rows in AG half0 table
XB = NCORES * H1                     # 24576 rows in AG half1 table


def _build_program(cA1, cB1, nA1x, nB1x, cA2, cB2, nA2x, nB2x):
    from concourse import bass, bacc, mybir, tile
    from concourse.masks import make_identity

    f32 = mybir.dt.float32
    bf16 = mybir.dt.bfloat16
    i16 = mybir.dt.int16
    AF = mybir.ActivationFunctionType
    OP = mybir.AluOpType

    TOT1 = sum(cA1) + sum(cB1)
    TOT2 = sum(cA2) + sum(cB2)

    nc = bacc.Bacc("TRN2", target_bir_lowering=False, debug=False,
                   num_devices=NCORES)

    xbf = nc.declare_dram_parameter("xbf", [N_NODES, IN_DIM], bf16, isOutput=False)
    idx1 = nc.declare_dram_parameter("idx1", [P, 8 * TOT1], i16, isOutput=False)
    meta1 = nc.declare_dram_parameter("meta1", [P, 2 * TOT1], f32, isOutput=False)
    idx2 = nc.declare_dram_parameter("idx2", [P, 8 * TOT2], i16, isOutput=False)
    meta2 = nc.declare_dram_parameter("meta2", [P, 2 * TOT2], f32, isOutput=False)
    bcolp = nc.declare_dram_parameter("bcolp", [P, NBLK], f32, isOutput=False)
    w1p = nc.declare_dram_parameter("w1p", [IN_DIM, HID], bf16, isOutput=False)
    w2p = nc.declare_dram_parameter("w2p", [HID, HID], bf16, isOutput=False)
    w3p = nc.declare_dram_parameter("w3p", [HID, HID], bf16, isOutput=False)
    bias = nc.declare_dram_parameter("bias", [P, 6], f32, isOutput=False)
    tsh = nc.declare_dram_parameter("tsh", [P, 6], f32, isOutput=False)
    lw1 = nc.declare_dram_parameter("lw1", [HID, HID], f32, isOutput=False)
    lb1c = nc.declare_dram_parameter("lb1c", [P, 2], f32, isOutput=False)
    lw2 = nc.declare_dram_parameter("lw2", [P, 2], f32, isOutput=False)
    lb2c = nc.declare_dram_parameter("lb2c", [1, 1], f32, isOutput=False)
    icnt = nc.declare_dram_parameter("icnt", [P, N_GRAPHS], f32, isOutput=False)
    out = nc.declare_dram_parameter("out", [1, N_GRAPHS], f32, isOutput=True)
    dbg_x1 = os.environ.get("GNN_DBG_X1")
    if dbg_x1:
        outx1a = nc.declare_dram_parameter("outx1a", [XA, HID], bf16, isOutput=True)
        outx1b = nc.declare_dram_parameter("outx1b", [XB, HID], bf16, isOutput=True)
        outx2a = nc.declare_dram_parameter("outx2a", [XA, HID], bf16, isOutput=True)
        outx2b = nc.declare_dram_parameter("outx2b", [XB, HID], bf16, isOutput=True)

    with tile.TileContext(nc) as tc:
        with tc.tile_pool(name="const", bufs=1) as cpool, \
             tc.tile_pool(name="rows", bufs=3) as rpool, \
             tc.tile_pool(name="smat", bufs=8) as spool, \
             tc.tile_pool(name="work", bufs=6) as wpool, \
             tc.tile_pool(name="tail", bufs=1) as tpool, \
             tc.tile_pool(name="resid", bufs=1) as residp, \
             tc.tile_pool(name="hrow", bufs=3) as hpool, \
             tc.tile_pool(name="psum", bufs=2, space="PSUM") as ppool, \
             tc.tile_pool(name="psumt", bufs=2, space="PSUM") as ppoolt, \
             tc.tile_pool(name="psump", bufs=1, space="PSUM") as ppoolp, \
             tc.tile_pool(name="dram", bufs=8, space="DRAM") as dpool:

            iota_i = cpool.tile([P, P], mybir.dt.int32, tag="ioi")
            nc.gpsimd.iota(iota_i[:], pattern=[[1, P]], base=0, channel_multiplier=0)
            iota_b = cpool.tile([P, P], bf16, tag="iob")
            nc.vector.tensor_copy(iota_b[:], iota_i[:])
            iota5_i = cpool.tile([P, N_GRAPHS], mybir.dt.int32, tag="io5i")
            nc.gpsimd.iota(iota5_i[:], pattern=[[1, N_GRAPHS]], base=0, channel_multiplier=0)
            iota5_f = cpool.tile([P, N_GRAPHS], f32, tag="io5f")
            nc.vector.tensor_copy(iota5_f[:], iota5_i[:])
            ident = cpool.tile([P, P], f32, tag="ident")
            make_identity(nc, ident[:])

            bias_t = cpool.tile([P, 6], f32, tag="bias")
            nc.sync.dma_start(out=bias_t[:], in_=bias[:, :])
            tsh_t = cpool.tile([P, 6], f32, tag="tsh")
            nc.sync.dma_start(out=tsh_t[:], in_=tsh[:, :])
            bcol_t = cpool.tile([P, NBLK], f32, tag="bcol")
            nc.sync.dma_start(out=bcol_t[:], in_=bcolp[:, :])

            w1_t = cpool.tile([IN_DIM, HID], bf16, tag="w1")
            nc.sync.dma_start(out=w1_t[:], in_=w1p[:, :])
            w2_t = [cpool.tile([P, HID], bf16, tag=f"w2_{k}", name=f"w2_{k}") for k in range(2)]
            w3_t = [cpool.tile([P, HID], bf16, tag=f"w3_{k}", name=f"w3_{k}") for k in range(2)]
            for k in range(2):
                nc.sync.dma_start(out=w2_t[k][:], in_=w2p[k * P:(k + 1) * P, :])
                nc.sync.dma_start(out=w3_t[k][:], in_=w3p[k * P:(k + 1) * P, :])

            idx1_t = cpool.tile([P, 8 * TOT1], i16, tag="idx1")
            nc.sync.dma_start(out=idx1_t[:], in_=idx1[:, :])
            meta1_t = cpool.tile([P, 2 * TOT1], f32, tag="meta1")
            nc.sync.dma_start(out=meta1_t[:], in_=meta1[:, :])
            idx2_t = cpool.tile([P, 8 * TOT2], i16, tag="idx2")
            nc.sync.dma_start(out=idx2_t[:], in_=idx2[:, :])
            meta2_t = cpool.tile([P, 2 * TOT2], f32, tag="meta2")
            nc.sync.dma_start(out=meta2_t[:], in_=meta2[:, :])

            hloc1 = dpool.tile([PADN, HID], bf16, tag="hloc1")
            hloc2 = dpool.tile([PADN, HID], bf16, tag="hloc2")
            xn1a = dpool.tile([XA, HID], bf16, tag="xn1a")
            xn1b = dpool.tile([XB, HID], bf16, tag="xn1b")
            xn2a = dpool.tile([XA, HID], bf16, tag="xn2a")
            xn2b = dpool.tile([XB, HID], bf16, tag="xn2b")
            prdram = dpool.tile([HID, N_GRAPHS], f32, tag="prd")
            ardram = dpool.tile([HID, N_GRAPHS], f32, tag="ard")

            CMAX = max(cA1[b] + cB1[b] for b in range(NBLK))
            CMAX = max(CMAX, max(cA2[b] + cB2[b] for b in range(NBLK)))
            for _mi in range(3):
                zt = rpool.tile([P, CMAX, HID], bf16, tag="xr", name=f"zt{_mi}")
                nc.vector.memset(zt[:], 0.0)
            resid = [[residp.tile([P, P], f32, tag=f"r{b}h{h}", name=f"r{b}h{h}") for h in range(2)]
                     for b in range(NBLK)]

            pooled_ps = [ppoolp.tile([P, N_GRAPHS], f32, tag=f"pool{h}", name=f"pool{h}")
                         for h in range(2)]

            groups = [list(range(NCORES))]

            def allgather_half(hloc, xna, xnb, half):
                if half == 0:
                    nc.gpsimd.collective_compute(
                        "AllGather", OP.bypass, replica_groups=groups,
                        ins=[hloc[0:H0, :]], outs=[xna[:, :]])
                else:
                    nc.gpsimd.collective_compute(
                        "AllGather", OP.bypass, replica_groups=groups,
                        ins=[hloc[H0:PADN, :]], outs=[xnb[:, :]])

            def layer(li, tabA, tabB, fdim, idx_t, meta_t, cAs, cBs, nAxs, nBxs,
                      wtiles, bc0, hloc, mid_cb=None):
                nf = fdim // P
                io = 0
                mo = 0
                nblk_dbg = int(os.environ.get("GNN_DBG_NBLK", str(NBLK)))
                for b in range(NBLK):
                    if b >= nblk_dbg:
                        break
                    ca, cb = cAs[b], cBs[b]
                    C = ca + cb
                    xr = rpool.tile([P, C, fdim], bf16, tag="xr")
                    # dma_gather tops out at 1024 indices (8 chunks) per call;
                    # exact num_idxs (not a 128-multiple) skips the pad slots
                    for tab, c0, c1, nx in ((tabA, 0, ca, nAxs[b]),
                                            (tabB, ca, C, nBxs[b])):
                        for s in range(c0, c1, 8):
                            e = min(s + 8, c1)
                            n = min(nx, (e - c0) * P) - (s - c0) * P
                            if n <= 0:
                                continue
                            nc.gpsimd.dma_gather(
                                xr[:, s:e, :], tab,
                                idx_t[:, io + 8 * s:io + 8 * e],
                                n, n, fdim)
                    io += 8 * C

                    aggT = [ppool.tile([P, P], f32, tag=f"agg{k}", name=f"aggps{k}")
                            for k in range(nf)]
                    for j in range(C):
                        smat = spool.tile([P, P], bf16, tag="smat")
                        nc.vector.tensor_scalar(
                            out=smat[:], in0=iota_b[:],
                            scalar1=meta_t[:, mo + j:mo + j + 1],
                            scalar2=meta_t[:, mo + C + j:mo + C + j + 1],
                            op0=OP.is_equal, op1=OP.mult)
                        for k in range(nf):
                            nc.tensor.matmul(
                                aggT[k][:], lhsT=xr[:, j, k * P:(k + 1) * P],
                                rhs=smat[:], start=(j == 0), stop=(j == C - 1))
                    mo += 2 * C

                    aggs = [wpool.tile([P, P], bf16, tag=f"aggs{k}", name=f"aggs{k}")
                            for k in range(nf)]
                    for k in range(nf):
                        nc.scalar.activation(aggs[k][:], aggT[k][:], AF.Copy)

                    hrow = hpool.tile([P, HID], bf16, tag="hrow")
                    for h in range(2):
                        hT_ps = ppoolt.tile([P, P], f32, tag="tmp")
                        for k in range(nf):
                            nc.tensor.matmul(
                                hT_ps[:], lhsT=wtiles[k][:, h * P:(h + 1) * P],
                                rhs=aggs[k][:], start=(k == 0), stop=(k == nf - 1))
                        hTs = wpool.tile([P, P], f32, tag=f"hTs{h}")
                        nc.scalar.activation(hTs[:], hT_ps[:], AF.Relu,
                                             bias=bias_t[:, bc0 + h:bc0 + h + 1])
                        if li == 0:
                            nc.scalar.activation(resid[b][h][:], hTs[:], AF.Identity,
                                                 bias=tsh_t[:, bc0 + h:bc0 + h + 1])
                        else:
                            u = wpool.tile([P, P], f32, tag=f"u{h}")
                            nc.scalar.activation(u[:], hTs[:], AF.Identity,
                                                 bias=tsh_t[:, bc0 + h:bc0 + h + 1])
                            nc.vector.tensor_tensor(
                                out=resid[b][h][:], in0=resid[b][h][:], in1=u[:],
                                op=OP.add)
                        tp_ps = ppoolt.tile([P, P], f32, tag="tmp")
                        nc.tensor.transpose(tp_ps[:], resid[b][h][:], ident[:])
                        nc.scalar.activation(hrow[:, h * P:(h + 1) * P], tp_ps[:], AF.Copy)

                    if hloc is not None:
                        nc.sync.dma_start(out=hloc[b * P:(b + 1) * P, :], in_=hrow[:])
                        if b == SPLIT_BLK - 1 and mid_cb is not None:
                            mid_cb()
                    else:
                        mblk = spool.tile([P, N_GRAPHS], bf16, tag="mblk")
                        nc.vector.tensor_scalar(
                            out=mblk[:], in0=iota5_f[:],
                            scalar1=bcol_t[:, b:b + 1], scalar2=None,
                            op0=OP.is_equal)
                        for h in range(2):
                            nc.tensor.matmul(
                                pooled_ps[h][:], lhsT=hrow[:, h * P:(h + 1) * P],
                                rhs=mblk[:], start=(b == 0), stop=(b == NBLK - 1))

            dbg_stop = os.environ.get("GNN_DBG_STOP", "full")
            layer(0, xbf[0:KS, :], xbf[KS:N_NODES, :], IN_DIM, idx1_t, meta1_t,
                  cA1, cB1, nA1x, nB1x, [w1_t], 0, hloc1,
                  mid_cb=(lambda: allgather_half(hloc1, xn1a, xn1b, 0))
                  if dbg_stop != "l1" else None)
            if dbg_stop != "l1":
                allgather_half(hloc1, xn1a, xn1b, 1)
            if dbg_stop in ("l2", "ag2", "full"):
                layer(1, xn1a[:, :], xn1b[:, :], HID, idx2_t, meta2_t,
                      cA2, cB2, nA2x, nB2x, w2_t, 2, hloc2,
                      mid_cb=(lambda: allgather_half(hloc2, xn2a, xn2b, 0))
                      if dbg_stop != "l2" else None)
            if dbg_stop in ("ag2", "full"):
                allgather_half(hloc2, xn2a, xn2b, 1)
            if dbg_stop == "full":
                layer(2, xn2a[:, :], xn2b[:, :], HID, idx2_t, meta2_t,
                      cA2, cB2, nA2x, nB2x, w3_t, 4, None)
            else:
                # dummy pooled so the tail still builds
                dummy = wpool.tile([P, N_GRAPHS], bf16, tag="dummy")
                nc.vector.tensor_copy(dummy[:], iota5_f[:])
                for h in range(2):
                    nc.tensor.matmul(pooled_ps[h][:], lhsT=dummy[:, 0:P],
                                     rhs=dummy[:], start=True, stop=True)

            # pooled partial sums -> DRAM -> AllReduce
            icnt_t = cpool.tile([P, N_GRAPHS], f32, tag="icnt")
            nc.sync.dma_start(out=icnt_t[:], in_=icnt[:, :])
            for h in range(2):
                ps = tpool.tile([P, N_GRAPHS], f32, tag=f"poolsb{h}")
                nc.scalar.activation(ps[:], pooled_ps[h][:], AF.Copy)
                nc.sync.dma_start(out=prdram[h * P:(h + 1) * P, :], in_=ps[:])
            nc.gpsimd.collective_compute(
                "AllReduce", OP.add, replica_groups=groups,
                ins=[prdram[:, :]], outs=[ardram[:, :]])

            # head: h1T[o,g] = relu(lw1.T @ (pooledT*icnt) + lb1); out = lw2.T @ h1T + lb2
            lw1_t = [cpool.tile([P, HID], f32, tag=f"lw1_{k}", name=f"lw1_{k}") for k in range(2)]
            lw2_t = cpool.tile([P, 2], f32, tag="lw2")
            lb1_t = cpool.tile([P, 2], f32, tag="lb1")
            lb2_t = cpool.tile([1, 1], f32, tag="lb2")
            for k in range(2):
                nc.sync.dma_start(out=lw1_t[k][:], in_=lw1[k * P:(k + 1) * P, :])
            nc.sync.dma_start(out=lw2_t[:], in_=lw2[:, :])
            nc.sync.dma_start(out=lb1_t[:], in_=lb1c[:, :])
            nc.sync.dma_start(out=lb2_t[:], in_=lb2c[:, :])

            par = []
            for k in range(2):
                pk = tpool.tile([P, N_GRAPHS], f32, tag=f"par{k}")
                nc.sync.dma_start(out=pk[:], in_=ardram[k * P:(k + 1) * P, :])
                pks = tpool.tile([P, N_GRAPHS], f32, tag=f"pars{k}")
                nc.vector.tensor_tensor(out=pks[:], in0=pk[:], in1=icnt_t[:], op=OP.mult)
                par.append(pks)
            h1s = []
            for h in range(2):
                h1_ps = ppool.tile([P, N_GRAPHS], f32, tag="agg0")
                for k in range(2):
                    nc.tensor.matmul(h1_ps[:], lhsT=lw1_t[k][:, h * P:(h + 1) * P],
                                     rhs=par[k][:], start=(k == 0), stop=(k == 1))
                h1sb = tpool.tile([P, N_GRAPHS], f32, tag=f"h1s{h}")
                nc.scalar.activation(h1sb[:], h1_ps[:], AF.Relu,
                                     bias=lb1_t[:, h:h + 1])
                h1s.append(h1sb)
            out_ps = ppool.tile([1, N_GRAPHS], f32, tag="agg1")
            for h in range(2):
                nc.tensor.matmul(out_ps[:], lhsT=lw2_t[:, h:h + 1],
                                 rhs=h1s[h][:], start=(h == 0), stop=(h == 1))
            out_sb = tpool.tile([1, N_GRAPHS], f32, tag="outs")
            nc.vector.tensor_scalar(out=out_sb[:], in0=out_ps[:],
                                    scalar1=lb2_t[0:1, 0:1], scalar2=None, op0=OP.add)
            nc.sync.dma_start(out=out[:, :], in_=out_sb[:])
            if dbg_x1:
                nc.sync.dma_start(out=outx1a[:, :], in_=xn1a[:, :])
                nc.sync.dma_start(out=outx1b[:, :], in_=xn1b[:, :])
                nc.sync.dma_start(out=outx2a[:, :], in_=xn2a[:, :])
                nc.sync.dma_start(out=outx2b[:, :], in_=xn2b[:, :])

    nc.compile()
    return nc


def _wrap16(flat):
    """int16 index list (len % 128 == 0) -> [128, len/16] wrap-16 layout,
    replicated across the 8 gpsimd core groups."""
    cols = len(flat) // 16
    return np.tile(flat.reshape(cols, 16).T, (8, 1)).astype(np.int16)


def _preprocess(edge_index, batch):
    src = np.asarray(edge_index[0], dtype=np.int64)
    tgt = np.asarray(edge_index[1], dtype=np.int64)
    batch = np.asarray(batch, dtype=np.int64)

    deg = np.bincount(tgt, minlength=N_NODES).astype(np.float64) + 1.0
    dinv = 1.0 / np.sqrt(deg)

    allsrc = np.concatenate([src, np.arange(N_NODES, dtype=np.int64)])
    alltgt = np.concatenate([tgt, np.arange(N_NODES, dtype=np.int64)])
    allw = (dinv[allsrc] * dinv[alltgt]).astype(np.float32)

    order = np.argsort(alltgt, kind="stable")
    allsrc, alltgt, allw = allsrc[order], alltgt[order], allw[order]

    coreid = alltgt // SHARD
    locid = alltgt - coreid * SHARD
    blkkey = coreid * NBLK + locid // P
    tloc = (locid % P).astype(np.float32)

    # remapped row ids in the split-allgathered activation table
    cs = allsrc // SHARD
    rs = allsrc - cs * SHARD
    rid = np.where(rs < H0, cs * H0 + rs, NCORES * H0 + cs * H1 + (rs - H0))

    blk_start = np.zeros(NBLK * NCORES + 1, dtype=np.int64)
    np.cumsum(np.bincount(blkkey, minlength=NBLK * NCORES), out=blk_start[1:])

    # per (core, block) A/B edge counts for both index spaces
    isB1 = allsrc >= KS
    isB2 = rid >= NCORES * H0

    def chunk_counts(isB):
        nA = np.zeros((NCORES, NBLK), np.int64)
        nB = np.zeros((NCORES, NBLK), np.int64)
        for c in range(NCORES):
            for b in range(NBLK):
                g = c * NBLK + b
                m = isB[blk_start[g]:blk_start[g + 1]]
                nB[c, b] = int(m.sum())
                nA[c, b] = len(m) - nB[c, b]
        cA = [int(math.ceil(nA[:, b].max() / P)) for b in range(NBLK)]
        cB = [int(math.ceil(nB[:, b].max() / P)) for b in range(NBLK)]
        nAx = [int(nA[:, b].max()) for b in range(NBLK)]
        nBx = [int(nB[:, b].max()) for b in range(NBLK)]
        return cA, cB, nAx, nBx

    cA1, cB1, nA1x, nB1x = chunk_counts(isB1)
    cA2, cB2, nA2x, nB2x = chunk_counts(isB2)
    TOT1 = sum(cA1) + sum(cB1)
    TOT2 = sum(cA2) + sum(cB2)

    per_core = []
    for c in range(NCORES):
        idx1 = np.zeros((P, 8 * TOT1), np.int16)
        meta1 = np.zeros((P, 2 * TOT1), np.float32)
        idx2 = np.zeros((P, 8 * TOT2), np.int16)
        meta2 = np.zeros((P, 2 * TOT2), np.float32)
        io1 = mo1 = io2 = mo2 = 0
        for b in range(NBLK):
            g = c * NBLK + b
            lo, hi = blk_start[g], blk_start[g + 1]
            s = allsrc[lo:hi]
            r = rid[lo:hi]
            t = tloc[lo:hi]
            w = allw[lo:hi]

            for (ids, isb, cA, cB, KSo, idxa, metaa, io, mo) in (
                    (s, isB1[lo:hi], cA1[b], cB1[b], KS, idx1, meta1, io1, mo1),
                    (r, isB2[lo:hi], cA2[b], cB2[b], NCORES * H0, idx2, meta2, io2, mo2)):
                C = cA + cB
                mA, mB = ~isb, isb
                na, nb = int(mA.sum()), int(mB.sum())
                ia = np.zeros(cA * P, np.int64)
                ib = np.zeros(cB * P, np.int64)
                ia[:na] = ids[mA]
                ib[:nb] = ids[mB] - KSo
                tt = np.zeros(C * P, np.float32)
                ww = np.zeros(C * P, np.float32)
                tt[:na] = t[mA]
                tt[cA * P:cA * P + nb] = t[mB]
                ww[:na] = w[mA]
                ww[cA * P:cA * P + nb] = w[mB]
                if cA:
                    idxa[:, io:io + 8 * cA] = _wrap16(ia)
                if cB:
                    idxa[:, io + 8 * cA:io + 8 * C] = _wrap16(ib)
                metaa[:, mo:mo + C] = tt.reshape(C, P).T
                metaa[:, mo + C:mo + 2 * C] = ww.reshape(C, P).T

            io1 += 8 * (cA1[b] + cB1[b])
            mo1 += 2 * (cA1[b] + cB1[b])
            io2 += 8 * (cA2[b] + cB2[b])
            mo2 += 2 * (cA2[b] + cB2[b])

        # batch column for pooling (pad rows -> -1)
        bvals = batch[c * SHARD:(c + 1) * SHARD].astype(np.float32)
        bpad = np.pad(bvals, (0, PADN - SHARD), constant_values=-1.0)
        bcol = bpad.reshape(NBLK, P).T.copy()  # [P, NBLK]
        per_core.append(dict(idx1=idx1, meta1=meta1, idx2=idx2, meta2=meta2,
                             bcolp=bcol))
    return per_core, (cA1, cB1, nA1x, nB1x), (cA2, cB2, nA2x, nB2x)


def kernel(**inputs):
    import ml_dtypes
    from concourse.bass_utils import run_bass_kernel_spmd

    x = np.asarray(inputs["x"], dtype=np.float32)
    edge_index = np.asarray(inputs["edge_index"])
    batch = np.asarray(inputs["batch"])

    per_core, cc1, cc2 = _preprocess(edge_index, batch)
    (cA1, cB1, nA1x, nB1x) = cc1
    (cA2, cB2, nA2x, nB2x) = cc2

    def g(k):
        return np.asarray(inputs[k], dtype=np.float32)

    params = {}
    params["xbf"] = x.astype(ml_dtypes.bfloat16)
    Ws = [g("W1"), g("W2"), g("W3")]
    bs = [g("b1"), g("b2"), g("b3")]
    bias = np.zeros((P, 6), np.float32)
    tshv = np.zeros((P, 6), np.float32)
    wp = []
    for i in range(3):
        gam, be, m, v = g(f"g{i+1}"), g(f"be{i+1}"), g(f"m{i+1}"), g(f"v{i+1}")
        s = gam / np.sqrt(v + BN_EPS)
        assert (s > 0).all(), "BN scale must be positive for relu folding"
        wp.append((Ws[i] * s[None, :]).astype(ml_dtypes.bfloat16))
        bp = (bs[i] * s).astype(np.float32)
        tv = (be - m * s).astype(np.float32)
        bias[:, 2 * i] = bp[:P]
        bias[:, 2 * i + 1] = bp[P:]
        tshv[:, 2 * i] = tv[:P]
        tshv[:, 2 * i + 1] = tv[P:]
    params["w1p"], params["w2p"], params["w3p"] = wp
    params["bias"] = bias
    params["tsh"] = tshv
    params["lw1"] = g("lw1")
    lb1 = g("lb1")
    lb1c = np.zeros((P, 2), np.float32)
    lb1c[:, 0] = lb1[:P]
    lb1c[:, 1] = lb1[P:]
    params["lb1c"] = lb1c
    lw2v = g("lw2").reshape(HID)
    params["lw2"] = np.stack([lw2v[:P], lw2v[P:]], axis=1).copy()
    params["lb2c"] = g("lb2").reshape(1, 1).astype(np.float32)
    cnt = np.bincount(np.asarray(batch, dtype=np.int64), minlength=N_GRAPHS)
    icnt = (1.0 / np.maximum(cnt, 1)).astype(np.float32)
    params["icnt"] = np.tile(icnt[None, :], (P, 1))

    nc = _build_program(cA1, cB1, nA1x, nB1x, cA2, cB2, nA2x, nB2x)

    in_maps = []
    for c in range(NCORES):
        m = dict(params)
        m.update(per_core[c])
        in_maps.append(m)

    res = run_bass_kernel_spmd(nc, in_maps, list(range(NCORES)),
                               trace=bool(os.environ.get("GNN_TRACE")))
    if os.environ.get("GNN_TRACE"):
        print("HW exec time:", res.exec_time_ns, "ns")
    global _last_results
    _last_results = res
    o = res.results[0]["out"]
    return np.asarray(o, dtype=np.float32).reshape(N_GRAPHS, OUT_DIM)


# revision 17
# speedup vs baseline: 3.0042x; 1.4969x over previous
"""GCN (3x GCNConv + BN + residual, mean-pool, MLP head) on 8 trn2 NeuronCores.

Sharding: nodes split contiguously across 8 cores (6250 each); each core owns
the edges whose TARGET lands in its shard (plus self-loops). Per layer, each
core aggregates source features over its incident edges (GCN normalization is
linear, so aggregate-then-transform), applies the folded linear+BN epilogue,
and the bf16 activations are AllGathered (split in two halves so the first
half overlaps the tail of the layer) so every core can gather arbitrary
source rows next layer. Per-graph pooled sums are AllReduced; the tiny MLP
head is computed redundantly on every core.

Hot path per (layer, 128-target block):
  dma_gather pulls all the block's source rows (one instruction per table
  half; int16 indices limit a gather table to 32768 rows, so tables are
  addressed as two base-offset halves);
  per 128-edge chunk, one fused DVE tensor_scalar builds the bf16 selection
  matrix S[e,t] = w'[e] * (tl[e]==t); PE accumulates aggT[f,t] += xr.T @ S in
  PSUM; PE transform hT[o,t] = W'.T @ aggT; scalar engine does
  relu(+bias')+BN-shift; Pool engine adds the residual; PE transposes back to
  row-major [t,o] for the bf16 activation table / pooling.
"""
import math
import os
import sys

import numpy as np

sys.path.insert(0, "/opt/trn_rl_repo")

N_NODES = 50000
N_EDGES = 800000
IN_DIM = 128
HID = 256
OUT_DIM = 1
N_GRAPHS = 512
BN_EPS = 1e-5
NCORES = 8
P = 128
SHARD = N_NODES // NCORES            # 6250
NBLK = (SHARD + P - 1) // P          # 49
PADN = NBLK * P                      # 6272 rows per core incl pad
SPLIT_BLK = 25                       # blocks per first AllGather half
H0 = SPLIT_BLK * P                   # 3200
H1 = PADN - H0                       # 3072
XROWS = PADN * NCORES                # 50176 rows in allgathered tables
KS = 32768                           # int16 gather-index limit (L1 x table)
XA = NCORES * H0                     # 25600 rows in AG half0 table
XB = NCORES * H1                     # 24576 rows in AG half1 table


def _build_program(cA1, cB1, nA1x, nB1x, cA2, cB2, nA2x, nB2x):
    from concourse import bass, bacc, mybir, tile
    from concourse.masks import make_identity

    f32 = mybir.dt.float32
    bf16 = mybir.dt.bfloat16
    i16 = mybir.dt.int16
    AF = mybir.ActivationFunctionType
    OP = mybir.AluOpType

    TOT1 = sum(cA1) + sum(cB1)
    TOT2 = sum(cA2) + sum(cB2)

    nc = bacc.Bacc("TRN2", target_bir_lowering=False, debug=False,
                   num_devices=NCORES, num_swdge_queues=4)

    xbf = nc.declare_dram_parameter("xbf", [N_NODES, IN_DIM], bf16, isOutput=False)
    idx1 = nc.declare_dram_parameter("idx1", [P, 8 * TOT1], i16, isOutput=False)
    meta1 = nc.declare_dram_parameter("meta1", [P, 2 * TOT1], f32, isOutput=False)
    idx2 = nc.declare_dram_parameter("idx2", [P, 8 * TOT2], i16, isOutput=False)
    meta2 = nc.declare_dram_parameter("meta2", [P, 2 * TOT2], f32, isOutput=False)
    bcolp = nc.declare_dram_parameter("bcolp", [P, NBLK], f32, isOutput=False)
    w1p = nc.declare_dram_parameter("w1p", [IN_DIM, HID], bf16, isOutput=False)
    w2p = nc.declare_dram_parameter("w2p", [HID, HID], bf16, isOutput=False)
    w3p = nc.declare_dram_parameter("w3p", [HID, HID], bf16, isOutput=False)
    bias = nc.declare_dram_parameter("bias", [P, 6], f32, isOutput=False)
    tsh = nc.declare_dram_parameter("tsh", [P, 6], f32, isOutput=False)
    lw1 = nc.declare_dram_parameter("lw1", [HID, HID], f32, isOutput=False)
    lb1c = nc.declare_dram_parameter("lb1c", [P, 2], f32, isOutput=False)
    lw2 = nc.declare_dram_parameter("lw2", [P, 2], f32, isOutput=False)
    lb2c = nc.declare_dram_parameter("lb2c", [1, 1], f32, isOutput=False)
    icnt = nc.declare_dram_parameter("icnt", [P, N_GRAPHS], f32, isOutput=False)
    out = nc.declare_dram_parameter("out", [1, N_GRAPHS], f32, isOutput=True)
    dbg_x1 = os.environ.get("GNN_DBG_X1")
    if dbg_x1:
        outx1a = nc.declare_dram_parameter("outx1a", [XA, HID], bf16, isOutput=True)
        outx1b = nc.declare_dram_parameter("outx1b", [XB, HID], bf16, isOutput=True)
        outx2a = nc.declare_dram_parameter("outx2a", [XA, HID], bf16, isOutput=True)
        outx2b = nc.declare_dram_parameter("outx2b", [XB, HID], bf16, isOutput=True)

    with tile.TileContext(nc) as tc:
        with tc.tile_pool(name="const", bufs=1) as cpool, \
             tc.tile_pool(name="rows", bufs=3) as rpool, \
             tc.tile_pool(name="smat", bufs=8) as spool, \
             tc.tile_pool(name="work", bufs=6) as wpool, \
             tc.tile_pool(name="tail", bufs=1) as tpool, \
             tc.tile_pool(name="resid", bufs=1) as residp, \
             tc.tile_pool(name="hrow", bufs=3) as hpool, \
             tc.tile_pool(name="psum", bufs=2, space="PSUM") as ppool, \
             tc.tile_pool(name="psumt", bufs=2, space="PSUM") as ppoolt, \
             tc.tile_pool(name="psump", bufs=1, space="PSUM") as ppoolp, \
             tc.tile_pool(name="dram", bufs=8, space="DRAM") as dpool:

            iota_i = cpool.tile([P, P], mybir.dt.int32, tag="ioi")
            nc.gpsimd.iota(iota_i[:], pattern=[[1, P]], base=0, channel_multiplier=0)
            iota_b = cpool.tile([P, P], bf16, tag="iob")
            nc.vector.tensor_copy(iota_b[:], iota_i[:])
            iota5_i = cpool.tile([P, N_GRAPHS], mybir.dt.int32, tag="io5i")
            nc.gpsimd.iota(iota5_i[:], pattern=[[1, N_GRAPHS]], base=0, channel_multiplier=0)
            iota5_f = cpool.tile([P, N_GRAPHS], f32, tag="io5f")
            nc.vector.tensor_copy(iota5_f[:], iota5_i[:])
            ident = cpool.tile([P, P], f32, tag="ident")
            make_identity(nc, ident[:])

            bias_t = cpool.tile([P, 6], f32, tag="bias")
            nc.sync.dma_start(out=bias_t[:], in_=bias[:, :])
            tsh_t = cpool.tile([P, 6], f32, tag="tsh")
            nc.sync.dma_start(out=tsh_t[:], in_=tsh[:, :])
            bcol_t = cpool.tile([P, NBLK], f32, tag="bcol")
            nc.sync.dma_start(out=bcol_t[:], in_=bcolp[:, :])

            w1_t = cpool.tile([IN_DIM, HID], bf16, tag="w1")
            nc.sync.dma_start(out=w1_t[:], in_=w1p[:, :])
            w2_t = [cpool.tile([P, HID], bf16, tag=f"w2_{k}", name=f"w2_{k}") for k in range(2)]
            w3_t = [cpool.tile([P, HID], bf16, tag=f"w3_{k}", name=f"w3_{k}") for k in range(2)]
            for k in range(2):
                nc.sync.dma_start(out=w2_t[k][:], in_=w2p[k * P:(k + 1) * P, :])
                nc.sync.dma_start(out=w3_t[k][:], in_=w3p[k * P:(k + 1) * P, :])

            idx1_t = cpool.tile([P, 8 * TOT1], i16, tag="idx1")
            nc.sync.dma_start(out=idx1_t[:], in_=idx1[:, :])
            meta1_t = cpool.tile([P, 2 * TOT1], f32, tag="meta1")
            nc.sync.dma_start(out=meta1_t[:], in_=meta1[:, :])
            idx2_t = cpool.tile([P, 8 * TOT2], i16, tag="idx2")
            nc.sync.dma_start(out=idx2_t[:], in_=idx2[:, :])
            meta2_t = cpool.tile([P, 2 * TOT2], f32, tag="meta2")
            nc.sync.dma_start(out=meta2_t[:], in_=meta2[:, :])

            hloc1 = dpool.tile([PADN, HID], bf16, tag="hloc1")
            hloc2 = dpool.tile([PADN, HID], bf16, tag="hloc2")
            xn1a = dpool.tile([XA, HID], bf16, tag="xn1a")
            xn1b = dpool.tile([XB, HID], bf16, tag="xn1b")
            xn2a = dpool.tile([XA, HID], bf16, tag="xn2a")
            xn2b = dpool.tile([XB, HID], bf16, tag="xn2b")
            prdram = dpool.tile([HID, N_GRAPHS], f32, tag="prd")
            ardram = dpool.tile([HID, N_GRAPHS], f32, tag="ard")

            CMAX = max(cA1[b] + cB1[b] for b in range(NBLK))
            CMAX = max(CMAX, max(cA2[b] + cB2[b] for b in range(NBLK)))
            for _mi in range(3):
                zt = rpool.tile([P, CMAX, HID], bf16, tag="xr", name=f"zt{_mi}")
                nc.vector.memset(zt[:], 0.0)
            resid = [[residp.tile([P, P], f32, tag=f"r{b}h{h}", name=f"r{b}h{h}") for h in range(2)]
                     for b in range(NBLK)]

            pooled_ps = [ppoolp.tile([P, N_GRAPHS], f32, tag=f"pool{h}", name=f"pool{h}")
                         for h in range(2)]

            groups = [list(range(NCORES))]

            def allgather_half(hloc, xna, xnb, half):
                if half == 0:
                    nc.gpsimd.collective_compute(
                        "AllGather", OP.bypass, replica_groups=groups,
                        ins=[hloc[0:H0, :]], outs=[xna[:, :]])
                else:
                    nc.gpsimd.collective_compute(
                        "AllGather", OP.bypass, replica_groups=groups,
                        ins=[hloc[H0:PADN, :]], outs=[xnb[:, :]])

            qctr = [0]

            def layer(li, tabA, tabB, fdim, idx_t, meta_t, cAs, cBs, nAxs, nBxs,
                      wtiles, bc0, hloc, mid_cb=None):
                nf = fdim // P
                io = 0
                mo = 0
                nblk_dbg = int(os.environ.get("GNN_DBG_NBLK", str(NBLK)))
                for b in range(NBLK):
                    if b >= nblk_dbg:
                        break
                    ca, cb = cAs[b], cBs[b]
                    C = ca + cb
                    xr = rpool.tile([P, C, fdim], bf16, tag="xr")
                    # dma_gather tops out at 1024 indices (8 chunks) per call;
                    # exact num_idxs (not a 128-multiple) skips the pad slots
                    for tab, c0, c1, nx in ((tabA, 0, ca, nAxs[b]),
                                            (tabB, ca, C, nBxs[b])):
                        for s in range(c0, c1, 8):
                            e = min(s + 8, c1)
                            n = min(nx, (e - c0) * P) - (s - c0) * P
                            if n <= 0:
                                continue
                            nc.gpsimd.dma_gather(
                                xr[:, s:e, :], tab,
                                idx_t[:, io + 8 * s:io + 8 * e],
                                n, n, fdim, queue_num=qctr[0] % 4)
                            qctr[0] += 1
                    io += 8 * C

                    aggT = [ppool.tile([P, P], f32, tag=f"agg{k}", name=f"aggps{k}")
                            for k in range(nf)]
                    for j in range(C):
                        smat = spool.tile([P, P], bf16, tag="smat")
                        nc.vector.tensor_scalar(
                            out=smat[:], in0=iota_b[:],
                            scalar1=meta_t[:, mo + j:mo + j + 1],
                            scalar2=meta_t[:, mo + C + j:mo + C + j + 1],
                            op0=OP.is_equal, op1=OP.mult)
                        for k in range(nf):
                            nc.tensor.matmul(
                                aggT[k][:], lhsT=xr[:, j, k * P:(k + 1) * P],
                                rhs=smat[:], start=(j == 0), stop=(j == C - 1))
                    mo += 2 * C

                    aggs = [wpool.tile([P, P], bf16, tag=f"aggs{k}", name=f"aggs{k}")
                            for k in range(nf)]
                    for k in range(nf):
                        nc.scalar.activation(aggs[k][:], aggT[k][:], AF.Copy)

                    hrow = hpool.tile([P, HID], bf16, tag="hrow")
                    for h in range(2):
                        hT_ps = ppoolt.tile([P, P], f32, tag="tmp")
                        for k in range(nf):
                            nc.tensor.matmul(
                                hT_ps[:], lhsT=wtiles[k][:, h * P:(h + 1) * P],
                                rhs=aggs[k][:], start=(k == 0), stop=(k == nf - 1))
                        hTs = wpool.tile([P, P], f32, tag=f"hTs{h}")
                        nc.scalar.activation(hTs[:], hT_ps[:], AF.Relu,
                                             bias=bias_t[:, bc0 + h:bc0 + h + 1])
                        if li == 0:
                            nc.scalar.activation(resid[b][h][:], hTs[:], AF.Identity,
                                                 bias=tsh_t[:, bc0 + h:bc0 + h + 1])
                        else:
                            u = wpool.tile([P, P], f32, tag=f"u{h}")
                            nc.scalar.activation(u[:], hTs[:], AF.Identity,
                                                 bias=tsh_t[:, bc0 + h:bc0 + h + 1])
                            nc.vector.tensor_tensor(
                                out=resid[b][h][:], in0=resid[b][h][:], in1=u[:],
                                op=OP.add)
                        tp_ps = ppoolt.tile([P, P], f32, tag="tmp")
                        nc.tensor.transpose(tp_ps[:], resid[b][h][:], ident[:])
                        nc.scalar.activation(hrow[:, h * P:(h + 1) * P], tp_ps[:], AF.Copy)

                    if hloc is not None:
                        nc.sync.dma_start(out=hloc[b * P:(b + 1) * P, :], in_=hrow[:])
                        if b == SPLIT_BLK - 1 and mid_cb is not None:
                            mid_cb()
                    else:
                        mblk = spool.tile([P, N_GRAPHS], bf16, tag="mblk")
                        nc.vector.tensor_scalar(
                            out=mblk[:], in0=iota5_f[:],
                            scalar1=bcol_t[:, b:b + 1], scalar2=None,
                            op0=OP.is_equal)
                        for h in range(2):
                            nc.tensor.matmul(
                                pooled_ps[h][:], lhsT=hrow[:, h * P:(h + 1) * P],
                                rhs=mblk[:], start=(b == 0), stop=(b == NBLK - 1))

            dbg_stop = os.environ.get("GNN_DBG_STOP", "full")
            layer(0, xbf[0:KS, :], xbf[KS:N_NODES, :], IN_DIM, idx1_t, meta1_t,
                  cA1, cB1, nA1x, nB1x, [w1_t], 0, hloc1,
                  mid_cb=(lambda: allgather_half(hloc1, xn1a, xn1b, 0))
                  if dbg_stop != "l1" else None)
            if dbg_stop != "l1":
                allgather_half(hloc1, xn1a, xn1b, 1)
            if dbg_stop in ("l2", "ag2", "full"):
                layer(1, xn1a[:, :], xn1b[:, :], HID, idx2_t, meta2_t,
                      cA2, cB2, nA2x, nB2x, w2_t, 2, hloc2,
                      mid_cb=(lambda: allgather_half(hloc2, xn2a, xn2b, 0))
                      if dbg_stop != "l2" else None)
            if dbg_stop in ("ag2", "full"):
                allgather_half(hloc2, xn2a, xn2b, 1)
            if dbg_stop == "full":
                layer(2, xn2a[:, :], xn2b[:, :], HID, idx2_t, meta2_t,
                      cA2, cB2, nA2x, nB2x, w3_t, 4, None)
            else:
                # dummy pooled so the tail still builds
                dummy = wpool.tile([P, N_GRAPHS], bf16, tag="dummy")
                nc.vector.tensor_copy(dummy[:], iota5_f[:])
                for h in range(2):
                    nc.tensor.matmul(pooled_ps[h][:], lhsT=dummy[:, 0:P],
                                     rhs=dummy[:], start=True, stop=True)

            # pooled partial sums -> DRAM -> AllReduce
            icnt_t = cpool.tile([P, N_GRAPHS], f32, tag="icnt")
            nc.sync.dma_start(out=icnt_t[:], in_=icnt[:, :])
            for h in range(2):
                ps = tpool.tile([P, N_GRAPHS], f32, tag=f"poolsb{h}")
                nc.scalar.activation(ps[:], pooled_ps[h][:], AF.Copy)
                nc.sync.dma_start(out=prdram[h * P:(h + 1) * P, :], in_=ps[:])
            nc.gpsimd.collective_compute(
                "AllReduce", OP.add, replica_groups=groups,
                ins=[prdram[:, :]], outs=[ardram[:, :]])

            # head: h1T[o,g] = relu(lw1.T @ (pooledT*icnt) + lb1); out = lw2.T @ h1T + lb2
            lw1_t = [cpool.tile([P, HID], f32, tag=f"lw1_{k}", name=f"lw1_{k}") for k in range(2)]
            lw2_t = cpool.tile([P, 2], f32, tag="lw2")
            lb1_t = cpool.tile([P, 2], f32, tag="lb1")
            lb2_t = cpool.tile([1, 1], f32, tag="lb2")
            for k in range(2):
                nc.sync.dma_start(out=lw1_t[k][:], in_=lw1[k * P:(k + 1) * P, :])
            nc.sync.dma_start(out=lw2_t[:], in_=lw2[:, :])
            nc.sync.dma_start(out=lb1_t[:], in_=lb1c[:, :])
            nc.sync.dma_start(out=lb2_t[:], in_=lb2c[:, :])

            par = []
            for k in range(2):
                pk = tpool.tile([P, N_GRAPHS], f32, tag=f"par{k}")
                nc.sync.dma_start(out=pk[:], in_=ardram[k * P:(k + 1) * P, :])
                pks = tpool.tile([P, N_GRAPHS], f32, tag=f"pars{k}")
                nc.vector.tensor_tensor(out=pks[:], in0=pk[:], in1=icnt_t[:], op=OP.mult)
                par.append(pks)
            h1s = []
            for h in range(2):
                h1_ps = ppool.tile([P, N_GRAPHS], f32, tag="agg0")
                for k in range(2):
                    nc.tensor.matmul(h1_ps[:], lhsT=lw1_t[k][:, h * P:(h + 1) * P],
                                     rhs=par[k][:], start=(k == 0), stop=(k == 1))
                h1sb = tpool.tile([P, N_GRAPHS], f32, tag=f"h1s{h}")
                nc.scalar.activation(h1sb[:], h1_ps[:], AF.Relu,
                                     bias=lb1_t[:, h:h + 1])
                h1s.append(h1sb)
            out_ps = ppool.tile([1, N_GRAPHS], f32, tag="agg1")
            for h in range(2):
                nc.tensor.matmul(out_ps[:], lhsT=lw2_t[:, h:h + 1],
                                 rhs=h1s[h][:], start=(h == 0), stop=(h == 1))
            out_sb = tpool.tile([1, N_GRAPHS], f32, tag="outs")
            nc.vector.tensor_scalar(out=out_sb[:], in0=out_ps[:],
                                    scalar1=lb2_t[0:1, 0:1], scalar2=None, op0=OP.add)
            nc.sync.dma_start(out=out[:, :], in_=out_sb[:])
            if dbg_x1:
                nc.sync.dma_start(out=outx1a[:, :], in_=xn1a[:, :])
                nc.sync.dma_start(out=outx1b[:, :], in_=xn1b[:, :])
                nc.sync.dma_start(out=outx2a[:, :], in_=xn2a[:, :])
                nc.sync.dma_start(out=outx2b[:, :], in_=xn2b[:, :])

    nc.compile()
    return nc


def _wrap16(flat):
    """int16 index list (len % 128 == 0) -> [128, len/16] wrap-16 layout,
    replicated across the 8 gpsimd core groups."""
    cols = len(flat) // 16
    return np.tile(flat.reshape(cols, 16).T, (8, 1)).astype(np.int16)


def _preprocess(edge_index, batch):
    src = np.asarray(edge_index[0], dtype=np.int64)
    tgt = np.asarray(edge_index[1], dtype=np.int64)
    batch = np.asarray(batch, dtype=np.int64)

    deg = np.bincount(tgt, minlength=N_NODES).astype(np.float64) + 1.0
    dinv = 1.0 / np.sqrt(deg)

    allsrc = np.concatenate([src, np.arange(N_NODES, dtype=np.int64)])
    alltgt = np.concatenate([tgt, np.arange(N_NODES, dtype=np.int64)])
    allw = (dinv[allsrc] * dinv[alltgt]).astype(np.float32)

    order = np.argsort(alltgt, kind="stable")
    allsrc, alltgt, allw = allsrc[order], alltgt[order], allw[order]

    coreid = alltgt // SHARD
    locid = alltgt - coreid * SHARD
    blkkey = coreid * NBLK + locid // P
    tloc = (locid % P).astype(np.float32)

    # remapped row ids in the split-allgathered activation table
    cs = allsrc // SHARD
    rs = allsrc - cs * SHARD
    rid = np.where(rs < H0, cs * H0 + rs, NCORES * H0 + cs * H1 + (rs - H0))

    blk_start = np.zeros(NBLK * NCORES + 1, dtype=np.int64)
    np.cumsum(np.bincount(blkkey, minlength=NBLK * NCORES), out=blk_start[1:])

    # per (core, block) A/B edge counts for both index spaces
    isB1 = allsrc >= KS
    isB2 = rid >= NCORES * H0

    def chunk_counts(isB):
        nA = np.zeros((NCORES, NBLK), np.int64)
        nB = np.zeros((NCORES, NBLK), np.int64)
        for c in range(NCORES):
            for b in range(NBLK):
                g = c * NBLK + b
                m = isB[blk_start[g]:blk_start[g + 1]]
                nB[c, b] = int(m.sum())
                nA[c, b] = len(m) - nB[c, b]
        cA = [int(math.ceil(nA[:, b].max() / P)) for b in range(NBLK)]
        cB = [int(math.ceil(nB[:, b].max() / P)) for b in range(NBLK)]
        nAx = [int(nA[:, b].max()) for b in range(NBLK)]
        nBx = [int(nB[:, b].max()) for b in range(NBLK)]
        return cA, cB, nAx, nBx

    cA1, cB1, nA1x, nB1x = chunk_counts(isB1)
    cA2, cB2, nA2x, nB2x = chunk_counts(isB2)
    TOT1 = sum(cA1) + sum(cB1)
    TOT2 = sum(cA2) + sum(cB2)

    per_core = []
    for c in range(NCORES):
        idx1 = np.zeros((P, 8 * TOT1), np.int16)
        meta1 = np.zeros((P, 2 * TOT1), np.float32)
        idx2 = np.zeros((P, 8 * TOT2), np.int16)
        meta2 = np.zeros((P, 2 * TOT2), np.float32)
        io1 = mo1 = io2 = mo2 = 0
        for b in range(NBLK):
            g = c * NBLK + b
            lo, hi = blk_start[g], blk_start[g + 1]
            s = allsrc[lo:hi]
            r = rid[lo:hi]
            t = tloc[lo:hi]
            w = allw[lo:hi]

            for (ids, isb, cA, cB, KSo, idxa, metaa, io, mo) in (
                    (s, isB1[lo:hi], cA1[b], cB1[b], KS, idx1, meta1, io1, mo1),
                    (r, isB2[lo:hi], cA2[b], cB2[b], NCORES * H0, idx2, meta2, io2, mo2)):
                C = cA + cB
                mA, mB = ~isb, isb
                na, nb = int(mA.sum()), int(mB.sum())
                ia = np.zeros(cA * P, np.int64)
                ib = np.zeros(cB * P, np.int64)
                ia[:na] = ids[mA]
                ib[:nb] = ids[mB] - KSo
                tt = np.zeros(C * P, np.float32)
                ww = np.zeros(C * P, np.float32)
                tt[:na] = t[mA]
                tt[cA * P:cA * P + nb] = t[mB]
                ww[:na] = w[mA]
                ww[cA * P:cA * P + nb] = w[mB]
                if cA:
                    idxa[:, io:io + 8 * cA] = _wrap16(ia)
                if cB:
                    idxa[:, io + 8 * cA:io + 8 * C] = _wrap16(ib)
                metaa[:, mo:mo + C] = tt.reshape(C, P).T
                metaa[:, mo + C:mo + 2 * C] = ww.reshape(C, P).T

            io1 += 8 * (cA1[b] + cB1[b])
            mo1 += 2 * (cA1[b] + cB1[b])
            io2 += 8 * (cA2[b] + cB2[b])
            mo2 += 2 * (cA2[b] + cB2[b])

        # batch column for pooling (pad rows -> -1)
        bvals = batch[c * SHARD:(c + 1) * SHARD].astype(np.float32)
        bpad = np.pad(bvals, (0, PADN - SHARD), constant_values=-1.0)
        bcol = bpad.reshape(NBLK, P).T.copy()  # [P, NBLK]
        per_core.append(dict(idx1=idx1, meta1=meta1, idx2=idx2, meta2=meta2,
                             bcolp=bcol))
    return per_core, (cA1, cB1, nA1x, nB1x), (cA2, cB2, nA2x, nB2x)


def kernel(**inputs):
    import ml_dtypes
    from concourse.bass_utils import run_bass_kernel_spmd

    x = np.asarray(inputs["x"], dtype=np.float32)
    edge_index = np.asarray(inputs["edge_index"])
    batch = np.asarray(inputs["batch"])

    per_core, cc1, cc2 = _preprocess(edge_index, batch)
    (cA1, cB1, nA1x, nB1x) = cc1
    (cA2, cB2, nA2x, nB2x) = cc2

    def g(k):
        return np.asarray(inputs[k], dtype=np.float32)

    params = {}
    params["xbf"] = x.astype(ml_dtypes.bfloat16)
    Ws = [g("W1"), g("W2"), g("W3")]
    bs = [g("b1"), g("b2"), g("b3")]
    bias = np.zeros((P, 6), np.float32)
    tshv = np.zeros((P, 6), np.float32)
    wp = []
    for i in range(3):
        gam, be, m, v = g(f"g{i+1}"), g(f"be{i+1}"), g(f"m{i+1}"), g(f"v{i+1}")
        s = gam / np.sqrt(v + BN_EPS)
        assert (s > 0).all(), "BN scale must be positive for relu folding"
        wp.append((Ws[i] * s[None, :]).astype(ml_dtypes.bfloat16))
        bp = (bs[i] * s).astype(np.float32)
        tv = (be - m * s).astype(np.float32)
        bias[:, 2 * i] = bp[:P]
        bias[:, 2 * i + 1] = bp[P:]
        tshv[:, 2 * i] = tv[:P]
        tshv[:, 2 * i + 1] = tv[P:]
    params["w1p"], params["w2p"], params["w3p"] = wp
    params["bias"] = bias
    params["tsh"] = tshv
    params["lw1"] = g("lw1")
    lb1 = g("lb1")
    lb1c = np.zeros((P, 2), np.float32)
    lb1c[:, 0] = lb1[:P]
    lb1c[:, 1] = lb1[P:]
    params["lb1c"] = lb1c
    lw2v = g("lw2").reshape(HID)
    params["lw2"] = np.stack([lw2v[:P], lw2v[P:]], axis=1).copy()
    params["lb2c"] = g("lb2").reshape(1, 1).astype(np.float32)
    cnt = np.bincount(np.asarray(batch, dtype=np.int64), minlength=N_GRAPHS)
    icnt = (1.0 / np.maximum(cnt, 1)).astype(np.float32)
    params["icnt"] = np.tile(icnt[None, :], (P, 1))

    nc = _build_program(cA1, cB1, nA1x, nB1x, cA2, cB2, nA2x, nB2x)

    in_maps = []
    for c in range(NCORES):
        m = dict(params)
        m.update(per_core[c])
        in_maps.append(m)

    res = run_bass_kernel_spmd(nc, in_maps, list(range(NCORES)),
                               trace=bool(os.environ.get("GNN_TRACE")))
    if os.environ.get("GNN_TRACE"):
        print("HW exec time:", res.exec_time_ns, "ns")
    global _last_results
    _last_results = res
    o = res.results[0]["out"]
    return np.asarray(o, dtype=np.float32).reshape(N_GRAPHS, OUT_DIM)
